# revision 12
# baseline (speedup 1.0000x reference)
"""GCN encoder (3-layer message passing + LayerNorm + mean pool) on 8 trn2 cores.

Strategy (see also spec sharding_hint):
  - Nodes partitioned 8 ways by id (dst-sharding). Self loops appended as
    ordinary edges. Per core, edges grouped by (dst-tile of 128, src-quarter)
    with a *shared* padded structure (max over cores) so a single SPMD NEFF
    serves all cores; per-core data = int16 gather indices + local-dst ids.
  - Per layer: dense z = h @ W on PE (activations PE-transposed on the fly),
    z scaled by deg^-1/2, cast bf16, AllGather'd in 4 node-quarter chunks
    (tables of 16384 rows keep dma_gather's int16 indices in range and let
    gathers of quarter q overlap the AllGather of quarter q+1).
  - Message passing: per-edge dma_gather of 512B rows into SBUF, segment-sum
    via one-hot(dst_local) matmuls accumulating in PSUM; bias added with a
    rank-1 (sqrt(deg) x b) matmul; epilogue relu+LayerNorm on ACT/DVE.
  - Mean pool: one-hot(batch) matmuls with an appended ones column, 33KB
    AllReduce, divide by max(count, 1).
"""

import math
import os
import sys

import numpy as np

for p in ("/opt/trn_rl_repo",):
    if p not in sys.path and os.path.isdir(p):
        sys.path.insert(0, p)

import ml_dtypes

BF16 = ml_dtypes.bfloat16


# ---------------------------------------------------------------------------
# configuration
# ---------------------------------------------------------------------------

class Cfg:
    def __init__(self, N=65536, E=1048576, NG=64, D_IN=256, D_HID=256, D_OUT=128,
                 NQ=4, GROUP=4, EPS=1e-5):
        self.N, self.E, self.NG = N, E, NG
        self.D_IN, self.D_HID, self.D_OUT = D_IN, D_HID, D_OUT
        self.EPS = EPS
        self.NCORES = 8
        self.P = 128
        self.NPC = N // self.NCORES                  # nodes per core
        self.NT = self.NPC // self.P                 # dst tiles per core
        self.NQ = NQ                                 # AllGather chunks (src quarters)
        self.QROWS = self.NPC // NQ                  # local rows per quarter
        self.TABROWS = self.QROWS * self.NCORES      # gather-table rows (int16 bound)
        assert self.TABROWS <= 32768
        self.GROUP = GROUP                           # dst tiles per gather group
        self.NGROUPS = self.NT // GROUP
        self.GPQ = self.NGROUPS // NQ                # groups per quarter
        assert self.NT % GROUP == 0 and self.NGROUPS % NQ == 0
        assert self.QROWS % self.P == 0
        self.DIMS = [(D_IN, D_HID), (D_HID, D_HID), (D_HID, D_OUT)]
        self.CWIN = 8                                # chunks per one-hot build
        self.DBG_NO_GATHER = False                   # crash bisection knobs
        self.DBG_LOCAL_TABLE = False
        self.MAX_CALL_IDX = 1024                     # split big dma_gather calls


# ---------------------------------------------------------------------------
# host-side edge preprocessing (pure integer index work)
# ---------------------------------------------------------------------------

class Prep:
    pass


def prep_edges(cfg, edge_index, batch):
    """Sort/partition edges, build the shared padded segment structure and the
    per-core int16 gather-index / local-dst arrays."""
    c = cfg
    P, NPC, NT, NQ, QROWS, GROUP = c.P, c.NPC, c.NT, c.NQ, c.QROWS, c.GROUP
    src = np.concatenate([np.asarray(edge_index[0], np.int64),
                          np.arange(c.N, dtype=np.int64)])
    dst = np.concatenate([np.asarray(edge_index[1], np.int64),
                          np.arange(c.N, dtype=np.int64)])
    deg = np.bincount(dst, minlength=c.N).astype(np.float32)

    core = dst // NPC
    q_of = (src % NPC) // QROWS                       # AllGather chunk of src
    gidx = (src // NPC) * QROWS + (src % NPC) - q_of * QROWS  # idx in table q

    per_core = []
    counts = np.zeros((c.NCORES, NT, NQ), np.int64)
    for ci in range(c.NCORES):
        m = core == ci
        s_q, s_g, d_l = q_of[m], gidx[m], dst[m] - ci * NPC
        t = d_l // P
        grp = t // GROUP
        order = np.lexsort((t, s_q, grp))             # group -> quarter -> tile
        s_q, s_g, d_l, t = s_q[order], s_g[order], d_l[order], t[order]
        cnt = np.bincount(t * NQ + s_q, minlength=NT * NQ).reshape(NT, NQ)
        counts[ci] = cnt
        per_core.append((s_q, s_g, d_l % P, t))

    pad = ((counts.max(axis=0) + P - 1) // P) * P     # shared [NT, NQ] padded sizes

    # stream layout: for g: for q: for t in g: seg(t, q)
    seg_base = np.zeros((NT, NQ), np.int64)           # slot offset of each segment
    call_base = np.zeros((c.NGROUPS, NQ), np.int64)   # slot offset of each gather call
    call_n = np.zeros((c.NGROUPS, NQ), np.int64)      # num_idxs per call
    group_base = np.zeros(c.NGROUPS, np.int64)
    pos = 0                                           # in slots (chunks of 128)
    for g in range(c.NGROUPS):
        group_base[g] = pos
        for q in range(NQ):
            call_base[g, q] = pos
            for t in range(g * GROUP, (g + 1) * GROUP):
                seg_base[t, q] = pos
                pos += pad[t, q] // P
            call_n[g, q] = (pos - call_base[g, q]) * P
    nchunks = pos
    totidx = nchunks * P

    pr = Prep()
    pr.deg = deg
    pr.pad, pr.seg_base = pad, seg_base
    pr.call_base, pr.call_n, pr.group_base = call_base, call_n, group_base
    pr.nchunks, pr.totidx = nchunks, totidx
    pr.smax = int((np.diff(np.append(group_base, nchunks))).max())
    # ordered chunk list per tile: global slot ids
    pr.tile_slots = [
        [int(s) for q in range(NQ)
         for s in range(seg_base[t, q], seg_base[t, q] + pad[t, q] // P)]
        for t in range(NT)
    ]

    pr.idx16, pr.dstloc = [], []
    for ci in range(c.NCORES):
        s_q, s_g, dl, t = per_core[ci]
        idx = np.zeros(totidx, np.int16)
        dloc = np.full(totidx, -1.0, np.float32)
        # per-(t,q) segment fill
        start = np.zeros((NT, NQ), np.int64)
        start[:, :] = seg_base * P
        # edges are already in (g, q, t) order; compute per-seg offsets
        key = t * NQ + s_q
        # stable positions within each segment
        seg_begin = (seg_base * P)[t, s_q]
        # rank within segment: since sorted by (grp, q, t), same-(t,q) edges
        # are contiguous; use cumulative count trick
        _, first_pos, inv = np.unique(key, return_index=True, return_inverse=True)
        offs = np.arange(len(key)) - first_pos[inv]
        ppos = seg_begin + offs
        idx[ppos] = s_g.astype(np.int16)
        dloc[ppos] = dl.astype(np.float32)
        cols = totidx // 16
        pr.idx16.append(np.tile(idx.reshape(cols, 16).T, (8, 1)))   # [128, cols]
        pr.dstloc.append(np.ascontiguousarray(
            dloc.reshape(nchunks, P).T.astype(BF16)))               # [128, nchunks]
    return pr


# ---------------------------------------------------------------------------
# bass program
# ---------------------------------------------------------------------------

def build_program(cfg, pr):
    import concourse.bass as bass
    import concourse.mybir as mybir
    import concourse.tile as tile
    from concourse import bacc
    from concourse.masks import make_identity
    from concourse._compat import axon_active

    c = cfg
    P, NT, NQ, NGROUPS, GROUP = c.P, c.NT, c.NQ, c.NGROUPS, c.GROUP
    f32, bf16, i16 = mybir.dt.float32, mybir.dt.bfloat16, mybir.dt.int16
    AF = mybir.ActivationFunctionType
    OP = mybir.AluOpType
    RG = [list(range(c.NCORES))]

    nc = bacc.Bacc("TRN2", target_bir_lowering=False, debug=False,
                   num_devices=c.NCORES)

    # ---- I/O ----
    x_in = nc.dram_tensor("x_c", [c.NPC, c.D_IN], f32, kind="ExternalInput")
    Wd, bd = [], []
    for li, (din, dout) in enumerate(c.DIMS):
        Wd.append(nc.dram_tensor(f"W{li + 1}c", [P, din // P, dout], bf16,
                                 kind="ExternalInput"))
        bd.append(nc.dram_tensor(f"b{li + 1}c", [1, dout], bf16,
                                 kind="ExternalInput"))
    gbe = [nc.dram_tensor(n, [c.D_HID], f32, kind="ExternalInput")
           for n in ("g1c", "be1c", "g2c", "be2c")]
    degc_d = nc.dram_tensor("deg_col", [P, NT], f32, kind="ExternalInput")
    degr_d = nc.dram_tensor("deg_row", [1, c.NPC], f32, kind="ExternalInput")
    batch_d = nc.dram_tensor("batch_col", [P, NT], bf16, kind="ExternalInput")
    idx_d = nc.dram_tensor("idx16", [P, pr.totidx // 16], i16, kind="ExternalInput")
    dstl_d = nc.dram_tensor("dstloc", [P, pr.nchunks], bf16, kind="ExternalInput")
    h_out = nc.dram_tensor("h_out", [c.NPC, c.D_OUT], f32, kind="ExternalOutput")
    pooled_out = nc.dram_tensor("pooled_out", [c.NG, c.D_OUT], f32,
                                kind="ExternalOutput")

    with tile.TileContext(nc, num_cores=c.NCORES) as tc:
      with tc.tile_pool(name="const", bufs=1) as cp, \
           tc.tile_pool(name="hrow", bufs=3) as hrp, \
           tc.tile_pool(name="ht", bufs=3) as htp, \
           tc.tile_pool(name="zs", bufs=4) as zsp, \
           tc.tile_pool(name="gbuf", bufs=2) as gbp, \
           tc.tile_pool(name="cwin", bufs=4) as cwp, \
           tc.tile_pool(name="l3", bufs=3) as l3p, \
           tc.tile_pool(name="spmm_ps", bufs=4, space="PSUM") as spp, \
           tc.tile_pool(name="z_ps", bufs=2, space="PSUM") as zpp, \
           tc.tile_pool(name="t_ps", bufs=2, space="PSUM") as tpp, \
           tc.tile_pool(name="dram", bufs=1, space="DRAM") as dr:

        # ---- constants ----
        Wt = []
        for li, (din, dout) in enumerate(c.DIMS):
            w = cp.tile([P, din // P, dout], bf16, name=f"Wt{li}")
            nc.sync.dma_start(out=w[:, :, :], in_=Wd[li][:, :, :])
            Wt.append(w)
        brow = []
        for li, (din, dout) in enumerate(c.DIMS):
            b = cp.tile([1, dout], bf16, name=f"brow{li}")
            nc.sync.dma_start(out=b[:, :], in_=bd[li][:, :])
            brow.append(b)
        gb_bc = []
        for gi, gt in enumerate(gbe):
            t_ = cp.tile([P, c.D_HID], f32, name=f"gbbc{gi}")
            bcast = bass.AP(tensor=gt, offset=0, ap=[[0, P], [1, c.D_HID]])
            nc.gpsimd.dma_start(out=t_[:, :], in_=bcast)
            gb_bc.append(t_)
        eps_t = cp.tile([P, 1], f32, name="eps_t")
        nc.vector.memset(eps_t[:, :], c.EPS)
        ident = cp.tile([P, P], f32, name="ident")
        make_identity(nc, ident[:, :])
        iota_i = cp.tile([P, c.CWIN * P], i16, name="iota_i")
        nc.gpsimd.iota(iota_i[:, :], pattern=[[0, c.CWIN], [1, P]], base=0,
                       channel_multiplier=0)
        iota_b = cp.tile([P, c.CWIN, P], bf16, name="iota_b")
        nc.vector.tensor_copy(out=iota_b[:, :, :],
                              in_=iota_i[:, :].rearrange("p (w d) -> p w d",
                                                         w=c.CWIN))
        deg_c = cp.tile([P, NT], f32, name="deg_c")
        nc.sync.dma_start(out=deg_c[:, :], in_=degc_d[:, :])
        dis_c = cp.tile([P, NT], f32, name="dis_c")
        nc.scalar.activation(out=dis_c[:, :], in_=deg_c[:, :], func=AF.Sqrt)
        nc.vector.reciprocal(out=dis_c[:, :], in_=dis_c[:, :])
        deg_r = cp.tile([1, c.NPC], f32, name="deg_r")
        nc.sync.dma_start(out=deg_r[:, :], in_=degr_d[:, :])
        invdis_r = cp.tile([1, c.NPC], bf16, name="invdis_r")
        nc.scalar.activation(out=invdis_r[:, :], in_=deg_r[:, :], func=AF.Sqrt)
        batch_c = cp.tile([P, NT], bf16, name="batch_c")
        nc.sync.dma_start(out=batch_c[:, :], in_=batch_d[:, :])
        idx_t = cp.tile([P, pr.totidx // 16], i16, name="idx_t")
        nc.sync.dma_start(out=idx_t[:, :], in_=idx_d[:, :])
        dstl_t = cp.tile([P, pr.nchunks], bf16, name="dstl_t")
        nc.sync.dma_start(out=dstl_t[:, :], in_=dstl_d[:, :])

        # ---- comm buffers ----
        zsin = [[dr.tile([c.QROWS, c.DIMS[li][1]], bf16, name=f"zsin{li}_{q}",
                         tag=f"zsin{li}_{q}")
                 for q in range(NQ)] for li in range(3)]
        # NOTE: dma_gather from the Shared scratchpad region faults the device
        # (NRT_EXEC_UNIT_UNRECOVERABLE) beyond small sizes; AllGather into a
        # Local internal tile works (bass warns about perf only).
        tab_space = "Shared" if False else "Local"
        zstab = [[dr.tile([c.TABROWS, c.DIMS[li][1]], bf16,
                          name=f"zstab{li}_{q}", tag=f"zstab{li}_{q}",
                          addr_space=tab_space)
                  for q in range(NQ)] for li in range(3)]
        pool_in = dr.tile([c.NG, 132], f32, name="pool_in", tag="pool_in")
        pool_ag = dr.tile([c.NG, 132], f32, name="pool_ag", tag="pool_ag",
                          addr_space="Shared")

        spmm_ps = [None] * NT      # live PSUM tile per dst tile

        def phase_a_tile(li, t_):
            """Produce zs(layer li, one node tile) from h(li-1); li=0 reads x."""
            din, dout = c.DIMS[li]
            if li == 0:
                hrow = hrp.tile([P, c.D_HID], f32, name="hrow", tag="hrow")
                nc.sync.dma_start(out=hrow[:, :din],
                                  in_=x_in[t_ * P:(t_ + 1) * P, :])
            else:
                ps = spmm_ps[t_]
                spmm_ps[t_] = None
                hrow = hrp.tile([P, c.D_HID], f32, name="hrow", tag="hrow")
                nc.scalar.activation(out=hrow[:, :din], in_=ps[:, :din],
                                     func=AF.Relu, scale=dis_c[:, t_:t_ + 1])
                st = hrp.tile([P, 6], f32, name="bnst", tag="bnst")
                nc.vector.bn_stats(out=st[:, :], in_=hrow[:, :din])
                mv = hrp.tile([P, 2], f32, name="bnmv", tag="bnmv")
                nc.vector.bn_aggr(out=mv[:, :], in_=st[:, :])
                rs = hrp.tile([P, 1], f32, name="rstd", tag="rstd")
                nc.scalar.activation(out=rs[:, :], in_=mv[:, 1:2], func=AF.Sqrt,
                                     bias=eps_t[:, :])
                nc.vector.reciprocal(out=rs[:, :], in_=rs[:, :])
                nc.vector.tensor_scalar(out=hrow[:, :din], in0=hrow[:, :din],
                                        scalar1=mv[:, 0:1], scalar2=rs[:, :],
                                        op0=OP.subtract, op1=OP.mult)
                gi = 2 * (li - 1)
                nc.vector.tensor_tensor(out=hrow[:, :din], in0=hrow[:, :din],
                                        in1=gb_bc[gi][:, :din], op=OP.mult)
                nc.vector.tensor_tensor(out=hrow[:, :din], in0=hrow[:, :din],
                                        in1=gb_bc[gi + 1][:, :din], op=OP.add)
            tps = tpp.tile([P, din // P, P], f32, name="tps", tag="tps")
            for ih in range(din // P):
                nc.tensor.transpose(out=tps[:, ih, :],
                                    in_=hrow[:, ih * P:(ih + 1) * P],
                                    identity=ident[:, :])
            hT = htp.tile([P, din // P, P], bf16, name="hT", tag="hT")
            nc.vector.tensor_copy(out=hT[:, :, :], in_=tps[:, :, :])
            zp = zpp.tile([P, c.D_HID], f32, name="zp", tag="zp")
            for ih in range(din // P):
                nc.tensor.matmul(out=zp[:, :dout], lhsT=hT[:, ih, :],
                                 rhs=Wt[li][:, ih, :], start=(ih == 0),
                                 stop=(ih == din // P - 1),
                                 skip_group_check=True)
            zst = zsp.tile([P, c.D_HID], bf16, name="zst", tag="zst")
            nc.scalar.activation(out=zst[:, :dout], in_=zp[:, :dout],
                                 func=AF.Copy, scale=dis_c[:, t_:t_ + 1])
            q = t_ // (NT // NQ)
            r0 = (t_ % (NT // NQ)) * P
            nc.sync.dma_start(out=zsin[li][q][r0:r0 + P, :], in_=zst[:, :dout])

        def ag_maybe(li, g):
            if (g + 1) % c.GPQ == 0:
                q = g // c.GPQ
                nc.gpsimd.collective_compute(
                    "AllGather", OP.bypass, replica_groups=RG,
                    ins=[zsin[li][q][:, :].opt()], outs=[zstab[li][q][:, :].opt()])

        def phase_b_group(li, g):
            """Gather + segment-sum matmuls for dst tiles of group g, layer li."""
            dout = c.DIMS[li][1]
            g0 = int(pr.group_base[g])
            ns = int(pr.group_base[g + 1] if g + 1 < NGROUPS else pr.nchunks) - g0
            gb = gbp.tile([P, pr.smax, dout], bf16, name="gb", tag="gb")
            if c.DBG_NO_GATHER:
                nc.vector.memset(gb[:, :ns, :], 0.001)
            else:
                for q in range(NQ):
                    n_all = int(pr.call_n[g, q])
                    src_tab = zstab[li][q]
                    for o in range(0, n_all, c.MAX_CALL_IDX):
                        n_idx = min(c.MAX_CALL_IDX, n_all - o)
                        b0 = int(pr.call_base[g, q]) - g0 + o // P
                        col0 = (int(pr.call_base[g, q]) * P + o) // 16
                        nc.gpsimd.dma_gather(
                            gb[:, b0:b0 + n_idx // P, :],
                            src_tab[:, :],
                            idx_t[:, col0:col0 + n_idx // 16],
                            n_idx, n_idx, dout)
            cw_of = {}
            for w0 in range(0, ns, c.CWIN):
                wl = min(c.CWIN, ns - w0)
                cw = cwp.tile([P, c.CWIN, P], bf16, name="cw", tag="cw")
                nc.vector.tensor_tensor(
                    out=cw[:, :wl, :],
                    in0=dstl_t[:, g0 + w0:g0 + w0 + wl, None].broadcast_to(
                        [P, wl, P]),
                    in1=iota_b[:, :wl, :], op=OP.is_equal)
                for j in range(wl):
                    cw_of[g0 + w0 + j] = (cw, j)
            for t_ in range(g * GROUP, (g + 1) * GROUP):
                ps = spp.tile([P, c.D_HID], f32, name="sps", tag="sps")
                spmm_ps[t_] = ps
                slots = pr.tile_slots[t_]
                for k, s in enumerate(slots):
                    cw, j = cw_of[s]
                    nc.tensor.matmul(out=ps[:, :dout], lhsT=cw[:, j, :],
                                     rhs=gb[:, s - g0, :], start=(k == 0),
                                     stop=False, skip_group_check=True)
                nc.tensor.matmul(out=ps[:, :dout],
                                 lhsT=invdis_r[:, t_ * P:(t_ + 1) * P],
                                 rhs=brow[li][:, :], start=False, stop=True,
                                 skip_group_check=True)

        # ---- bootstrap: phase A of layer 1 from x ----
        for g in range(NGROUPS):
            for t_ in range(g * GROUP, (g + 1) * GROUP):
                phase_a_tile(0, t_)
            ag_maybe(0, g)

        pool_ps = None
        for li in range(3):
            dout = c.DIMS[li][1]
            if li == 2:
                pool_ps = zpp.tile([c.NG, 132], f32, name="poolps", tag="zp")
            for g in range(NGROUPS):
                phase_b_group(li, g)
                if li < 2:
                    for t_ in range(g * GROUP, (g + 1) * GROUP):
                        phase_a_tile(li + 1, t_)
                    ag_maybe(li + 1, g)
                else:
                    for t_ in range(g * GROUP, (g + 1) * GROUP):
                        ps = spmm_ps[t_]
                        spmm_ps[t_] = None
                        h3 = hrp.tile([P, c.D_HID], f32, name="hrow", tag="hrow")
                        nc.scalar.activation(out=h3[:, :dout], in_=ps[:, :dout],
                                             func=AF.Copy,
                                             scale=dis_c[:, t_:t_ + 1])
                        nc.sync.dma_start(out=h_out[t_ * P:(t_ + 1) * P, :],
                                          in_=h3[:, :dout])
                        h3b = l3p.tile([P, dout + 4], bf16, name="h3b", tag="h3b")
                        nc.vector.tensor_copy(out=h3b[:, :dout], in_=h3[:, :dout])
                        nc.vector.memset(h3b[:, dout:dout + 1], 1.0)
                        oh = l3p.tile([P, c.NG], bf16, name="oh", tag="oh")
                        nc.vector.tensor_tensor(
                            out=oh[:, :],
                            in0=batch_c[:, t_:t_ + 1].broadcast_to([P, c.NG]),
                            in1=iota_b[:, 0, :c.NG], op=OP.is_equal)
                        nc.tensor.matmul(out=pool_ps[:, :dout + 1],
                                         lhsT=oh[:, :], rhs=h3b[:, :dout + 1],
                                         start=(t_ == 0), stop=(t_ == NT - 1),
                                         skip_group_check=True)

        # ---- pooled tail ----
        psb = cp.tile([c.NG, 132], f32, name="psb")
        nc.vector.memset(psb[:, :], 0.0)
        nc.vector.tensor_copy(out=psb[:, :c.D_OUT + 1],
                              in_=pool_ps[:, :c.D_OUT + 1])
        nc.sync.dma_start(out=pool_in[:, :], in_=psb[:, :])
        nc.gpsimd.collective_compute(
            "AllReduce", mybir_add(nc), replica_groups=RG,
            ins=[pool_in[:, :].opt()], outs=[pool_ag[:, :].opt()])
        pres = cp.tile([c.NG, 132], f32, name="pres")
        nc.sync.dma_start(out=pres[:, :], in_=pool_ag[:, :])
        cnt = cp.tile([c.NG, 1], f32, name="cnt")
        nc.vector.tensor_scalar(out=cnt[:, :], in0=pres[:, c.D_OUT:c.D_OUT + 1],
                                scalar1=1.0, scalar2=None, op0=OP.max)
        nc.vector.reciprocal(out=cnt[:, :], in_=cnt[:, :])
        pooled = cp.tile([c.NG, c.D_OUT], f32, name="pooled")
        nc.vector.tensor_scalar(out=pooled[:, :], in0=pres[:, :c.D_OUT],
                                scalar1=cnt[:, :], scalar2=None, op0=OP.mult)
        nc.sync.dma_start(out=pooled_out[:, :], in_=pooled[:, :])

    nc.compile()
    return nc


def mybir_add(nc):
    import concourse.mybir as mybir
    return mybir.AluOpType.add


# ---------------------------------------------------------------------------
# per-core input maps
# ---------------------------------------------------------------------------

def make_in_maps(cfg, pr, inputs):
    c = cfg
    x = np.asarray(inputs["x"], np.float32)
    batch = np.asarray(inputs["batch"], np.int64)
    Ws = [np.asarray(inputs[k], np.float32) for k in ("W1", "W2", "W3")]
    bs = [np.asarray(inputs[k], np.float32) for k in ("b1", "b2", "b3")]
    gs = [np.asarray(inputs[k], np.float32) for k in ("g1", "be1", "g2", "be2")]

    maps = []
    for ci in range(c.NCORES):
        sl = slice(ci * c.NPC, (ci + 1) * c.NPC)
        m = {"x_c": np.ascontiguousarray(x[sl])}
        for li in range(3):
            W = Ws[li]
            din, dout = c.DIMS[li]
            m[f"W{li + 1}c"] = np.ascontiguousarray(
                W.reshape(din // c.P, c.P, dout).transpose(1, 0, 2).astype(BF16))
            m[f"b{li + 1}c"] = bs[li].reshape(1, -1).astype(BF16)
        for gi, n in enumerate(("g1c", "be1c", "g2c", "be2c")):
            m[n] = np.ascontiguousarray(gs[gi])
        dg = pr.deg[sl]
        m["deg_col"] = np.ascontiguousarray(dg.reshape(c.NT, c.P).T)
        m["deg_row"] = np.ascontiguousarray(dg.reshape(1, c.NPC))
        m["batch_col"] = np.ascontiguousarray(
            batch[sl].reshape(c.NT, c.P).T.astype(BF16))
        m["idx16"] = np.ascontiguousarray(pr.idx16[ci])
        m["dstloc"] = np.ascontiguousarray(pr.dstloc[ci])
        maps.append(m)
    return maps


# ---------------------------------------------------------------------------
# entry points
# ---------------------------------------------------------------------------

_CACHE = {}


def _build(inputs, cfg=None):
    cfg = cfg or Cfg()
    key = (cfg.N, cfg.E, cfg.NG, cfg.NQ, cfg.GROUP)
    if key not in _CACHE:
        pr = prep_edges(cfg, np.asarray(inputs["edge_index"], np.int64),
                        np.asarray(inputs["batch"], np.int64))
        nc = build_program(cfg, pr)
        _CACHE[key] = (cfg, pr, nc)
    cfg, pr, nc = _CACHE[key]
    in_maps = make_in_maps(cfg, pr, inputs)
    return cfg, pr, nc, in_maps


def _run(inputs, trace=False, cfg=None):
    from concourse import bass_utils
    cfg, pr, nc, in_maps = _build(inputs, cfg)
    res = bass_utils.run_bass_kernel_spmd(
        nc, in_maps, core_ids=list(range(cfg.NCORES)), trace=False)
    h = np.concatenate([r["h_out"] for r in res.results], axis=0)
    pooled = res.results[0]["pooled_out"]
    return (h.astype(np.float32), pooled.astype(np.float32)), res


def bench(inputs, iters=10, cfg=None):
    """Build the sharded PJRT executable once, keep inputs device-resident,
    and wall-clock repeated executions. Returns ((h, pooled), best_ns)."""
    import time

    import jax
    from jax.sharding import Mesh, NamedSharding, PartitionSpec
    try:
        from jax.experimental.shard_map import shard_map
    except ImportError:
        from jax.sharding import shard_map
    import concourse.mybir as mybir
    from concourse import bass2jax

    cfg, pr, nc, in_maps = _build(inputs, cfg)
    n_cores = cfg.NCORES
    bass2jax.install_neuronx_cc_hook()

    partition_name = (nc.partition_id_tensor.name
                      if nc.partition_id_tensor else None)
    in_names, out_names, out_avals, zero_outs = [], [], [], []
    for alloc in nc.m.functions[0].allocations:
        if not isinstance(alloc, mybir.MemoryLocationSet):
            continue
        name = alloc.memorylocations[0].name
        if alloc.kind == "ExternalInput":
            if name != partition_name:
                in_names.append(name)
        elif alloc.kind == "ExternalOutput":
            shape = tuple(alloc.tensor_shape)
            dtype = mybir.dt.np(alloc.dtype)
            out_names.append(name)
            out_avals.append(jax.core.ShapedArray(shape, dtype))
            zero_outs.append(np.zeros(shape, dtype))
    n_params = len(in_names)
    all_in_names = in_names + out_names
    if partition_name is not None:
        all_in_names = all_in_names + [partition_name]

    def _exec_once(ins, zeros):
        operands = list(ins) + list(zeros)
        if partition_name is not None:
            operands.append(bass2jax.partition_id_tensor())
        outs = bass2jax._bass_exec_p.bind(
            *operands,
            out_avals=tuple(out_avals),
            in_names=tuple(all_in_names),
            out_names=tuple(out_names),
            lowering_input_output_aliases=(),
            sim_require_finite=True,
            sim_require_nnan=True,
            nc=nc,
        )
        return tuple(outs)

    chain = int(os.environ.get("BENCH_CHAIN", "1"))

    def _body(*args):
        ins, zeros = args[:n_params], args[n_params:]
        outs = _exec_once(ins, zeros)
        for _ in range(chain - 1):
            outs = _exec_once(ins, outs)
        return outs

    devices = jax.devices()[:n_cores]
    mesh = Mesh(np.asarray(devices), ("core",))
    spec = PartitionSpec("core")
    sharded = jax.jit(
        shard_map(_body, mesh=mesh, in_specs=(spec,) * (n_params + len(out_names)),
                  out_specs=(spec,) * len(out_names), check_rep=False),
        keep_unused=True)
    sh = NamedSharding(mesh, spec)
    concat_in = [
        jax.device_put(
            np.concatenate([np.asarray(in_maps[c][n]) for c in range(n_cores)],
                           axis=0), sh)
        for n in in_names
    ]
    concat_zeros = [
        jax.device_put(np.zeros((n_cores * z.shape[0], *z.shape[1:]), z.dtype), sh)
        for z in zero_outs
    ]
    out = sharded(*concat_in, *concat_zeros)
    jax.block_until_ready(out)
    times = []
    for _ in range(iters):
        t0 = time.perf_counter()
        out = sharded(*concat_in, *concat_zeros)
        jax.block_until_ready(out)
        times.append(time.perf_counter() - t0)
    best_ns = int(min(times) * 1e9)
    outs = {n: np.asarray(out[i]) for i, n in enumerate(out_names)}
    h = outs["h_out"].reshape(n_cores, cfg.NPC, cfg.D_OUT).reshape(-1, cfg.D_OUT)
    pooled = outs["pooled_out"].reshape(n_cores, cfg.NG, cfg.D_OUT)[0]
    print("bench times (ms):", [f"{t * 1e3:.3f}" for t in times])
    return (h.astype(np.float32), pooled.astype(np.float32)), best_ns


def kernel(**inputs):
    (h, pooled), _ = _run(inputs, trace=False)
    return h, pooled


# revision 14
# speedup vs baseline: 1.3717x; 1.3717x over previous
"""GCN encoder (3-layer message passing + LayerNorm + mean pool) on 8 trn2 cores.

Strategy (see also spec sharding_hint):
  - Nodes partitioned 8 ways by id (dst-sharding). Self loops appended as
    ordinary edges. Per core, edges grouped by (dst-tile of 128, src-quarter)
    with a *shared* padded structure (max over cores) so a single SPMD NEFF
    serves all cores; per-core data = int16 gather indices + local-dst ids.
  - Per layer: dense z = h @ W on PE (activations PE-transposed on the fly),
    z scaled by deg^-1/2, cast bf16, AllGather'd in 4 node-quarter chunks
    (tables of 16384 rows keep dma_gather's int16 indices in range and let
    gathers of quarter q overlap the AllGather of quarter q+1).
  - Message passing: per-edge dma_gather of 512B rows into SBUF, segment-sum
    via one-hot(dst_local) matmuls accumulating in PSUM; bias added with a
    rank-1 (sqrt(deg) x b) matmul; epilogue relu+LayerNorm on ACT/DVE.
  - Mean pool: one-hot(batch) matmuls with an appended ones column, 33KB
    AllReduce, divide by max(count, 1).
"""

import math
import os
import sys

import numpy as np

for p in ("/opt/trn_rl_repo",):
    if p not in sys.path and os.path.isdir(p):
        sys.path.insert(0, p)

import ml_dtypes

BF16 = ml_dtypes.bfloat16


# ---------------------------------------------------------------------------
# configuration
# ---------------------------------------------------------------------------

class Cfg:
    def __init__(self, N=65536, E=1048576, NG=64, D_IN=256, D_HID=256, D_OUT=128,
                 NQ=4, GROUP=4, EPS=1e-5):
        self.N, self.E, self.NG = N, E, NG
        self.D_IN, self.D_HID, self.D_OUT = D_IN, D_HID, D_OUT
        self.EPS = EPS
        self.NCORES = 8
        self.P = 128
        self.NPC = N // self.NCORES                  # nodes per core
        self.NT = self.NPC // self.P                 # dst tiles per core
        self.NQ = NQ                                 # AllGather chunks (src quarters)
        self.QROWS = self.NPC // NQ                  # local rows per quarter
        self.TABROWS = self.QROWS * self.NCORES      # gather-table rows (int16 bound)
        assert self.TABROWS <= 32768
        self.GROUP = GROUP                           # dst tiles per gather group
        self.NGROUPS = self.NT // GROUP
        self.GPQ = self.NGROUPS // NQ                # groups per quarter
        assert self.NT % GROUP == 0 and self.NGROUPS % NQ == 0
        assert self.QROWS % self.P == 0
        self.DIMS = [(D_IN, D_HID), (D_HID, D_HID), (D_HID, D_OUT)]
        self.CWIN = 8                                # chunks per one-hot build
        self.DBG_NO_GATHER = False                   # crash bisection knobs
        self.DBG_LOCAL_TABLE = False
        self.MAX_CALL_IDX = 1 << 30                  # split big dma_gather calls
        self.SINGLE_PACKET = False  # single_packet=True serializes + faults >2k idx


# ---------------------------------------------------------------------------
# host-side edge preprocessing (pure integer index work)
# ---------------------------------------------------------------------------

class Prep:
    pass


def prep_edges(cfg, edge_index, batch):
    """Sort/partition edges, build the shared padded segment structure and the
    per-core int16 gather-index / local-dst arrays."""
    c = cfg
    P, NPC, NT, NQ, QROWS, GROUP = c.P, c.NPC, c.NT, c.NQ, c.QROWS, c.GROUP
    src = np.concatenate([np.asarray(edge_index[0], np.int64),
                          np.arange(c.N, dtype=np.int64)])
    dst = np.concatenate([np.asarray(edge_index[1], np.int64),
                          np.arange(c.N, dtype=np.int64)])
    deg = np.bincount(dst, minlength=c.N).astype(np.float32)

    core = dst // NPC
    q_of = (src % NPC) // QROWS                       # AllGather chunk of src
    gidx = (src // NPC) * QROWS + (src % NPC) - q_of * QROWS  # idx in table q

    per_core = []
    counts = np.zeros((c.NCORES, NT, NQ), np.int64)
    for ci in range(c.NCORES):
        m = core == ci
        s_q, s_g, d_l = q_of[m], gidx[m], dst[m] - ci * NPC
        t = d_l // P
        grp = t // GROUP
        order = np.lexsort((t, s_q, grp))             # group -> quarter -> tile
        s_q, s_g, d_l, t = s_q[order], s_g[order], d_l[order], t[order]
        cnt = np.bincount(t * NQ + s_q, minlength=NT * NQ).reshape(NT, NQ)
        counts[ci] = cnt
        per_core.append((s_q, s_g, d_l % P, t))

    pad = ((counts.max(axis=0) + P - 1) // P) * P     # shared [NT, NQ] padded sizes

    # stream layout: for g: for q: for t in g: seg(t, q)
    seg_base = np.zeros((NT, NQ), np.int64)           # slot offset of each segment
    call_base = np.zeros((c.NGROUPS, NQ), np.int64)   # slot offset of each gather call
    call_n = np.zeros((c.NGROUPS, NQ), np.int64)      # num_idxs per call
    group_base = np.zeros(c.NGROUPS, np.int64)
    pos = 0                                           # in slots (chunks of 128)
    for g in range(c.NGROUPS):
        group_base[g] = pos
        for q in range(NQ):
            call_base[g, q] = pos
            for t in range(g * GROUP, (g + 1) * GROUP):
                seg_base[t, q] = pos
                pos += pad[t, q] // P
            call_n[g, q] = (pos - call_base[g, q]) * P
    nchunks = pos
    totidx = nchunks * P

    pr = Prep()
    pr.deg = deg
    pr.pad, pr.seg_base = pad, seg_base
    pr.call_base, pr.call_n, pr.group_base = call_base, call_n, group_base
    pr.nchunks, pr.totidx = nchunks, totidx
    pr.smax = int((np.diff(np.append(group_base, nchunks))).max())
    # ordered chunk list per tile: global slot ids
    pr.tile_slots = [
        [int(s) for q in range(NQ)
         for s in range(seg_base[t, q], seg_base[t, q] + pad[t, q] // P)]
        for t in range(NT)
    ]

    pr.idx16, pr.dstloc = [], []
    for ci in range(c.NCORES):
        s_q, s_g, dl, t = per_core[ci]
        idx = np.zeros(totidx, np.int16)
        dloc = np.full(totidx, -1.0, np.float32)
        # per-(t,q) segment fill
        start = np.zeros((NT, NQ), np.int64)
        start[:, :] = seg_base * P
        # edges are already in (g, q, t) order; compute per-seg offsets
        key = t * NQ + s_q
        # stable positions within each segment
        seg_begin = (seg_base * P)[t, s_q]
        # rank within segment: since sorted by (grp, q, t), same-(t,q) edges
        # are contiguous; use cumulative count trick
        _, first_pos, inv = np.unique(key, return_index=True, return_inverse=True)
        offs = np.arange(len(key)) - first_pos[inv]
        ppos = seg_begin + offs
        idx[ppos] = s_g.astype(np.int16)
        dloc[ppos] = dl.astype(np.float32)
        cols = totidx // 16
        pr.idx16.append(np.tile(idx.reshape(cols, 16).T, (8, 1)))   # [128, cols]
        pr.dstloc.append(np.ascontiguousarray(
            dloc.reshape(nchunks, P).T.astype(BF16)))               # [128, nchunks]
    return pr


# ---------------------------------------------------------------------------
# bass program
# ---------------------------------------------------------------------------

def build_program(cfg, pr):
    import concourse.bass as bass
    import concourse.mybir as mybir
    import concourse.tile as tile
    from concourse import bacc
    from concourse.masks import make_identity
    from concourse._compat import axon_active

    c = cfg
    P, NT, NQ, NGROUPS, GROUP = c.P, c.NT, c.NQ, c.NGROUPS, c.GROUP
    f32, bf16, i16 = mybir.dt.float32, mybir.dt.bfloat16, mybir.dt.int16
    AF = mybir.ActivationFunctionType
    OP = mybir.AluOpType
    RG = [list(range(c.NCORES))]

    nc = bacc.Bacc("TRN2", target_bir_lowering=False, debug=False,
                   num_devices=c.NCORES)

    # ---- I/O ----
    x_in = nc.dram_tensor("x_c", [c.NPC, c.D_IN], f32, kind="ExternalInput")
    Wd, bd = [], []
    for li, (din, dout) in enumerate(c.DIMS):
        Wd.append(nc.dram_tensor(f"W{li + 1}c", [P, din // P, dout], bf16,
                                 kind="ExternalInput"))
        bd.append(nc.dram_tensor(f"b{li + 1}c", [1, dout], bf16,
                                 kind="ExternalInput"))
    gbe = [nc.dram_tensor(n, [c.D_HID], f32, kind="ExternalInput")
           for n in ("g1c", "be1c", "g2c", "be2c")]
    degc_d = nc.dram_tensor("deg_col", [P, NT], f32, kind="ExternalInput")
    degr_d = nc.dram_tensor("deg_row", [1, c.NPC], f32, kind="ExternalInput")
    batch_d = nc.dram_tensor("batch_col", [P, NT], bf16, kind="ExternalInput")
    idx_d = nc.dram_tensor("idx16", [P, pr.totidx // 16], i16, kind="ExternalInput")
    dstl_d = nc.dram_tensor("dstloc", [P, pr.nchunks], bf16, kind="ExternalInput")
    h_out = nc.dram_tensor("h_out", [c.NPC, c.D_OUT], f32, kind="ExternalOutput")
    pooled_out = nc.dram_tensor("pooled_out", [c.NG, c.D_OUT], f32,
                                kind="ExternalOutput")

    with tile.TileContext(nc, num_cores=c.NCORES) as tc:
      with tc.tile_pool(name="const", bufs=1) as cp, \
           tc.tile_pool(name="hrow", bufs=3) as hrp, \
           tc.tile_pool(name="ht", bufs=3) as htp, \
           tc.tile_pool(name="zs", bufs=4) as zsp, \
           tc.tile_pool(name="gbuf", bufs=2) as gbp, \
           tc.tile_pool(name="cwin", bufs=4) as cwp, \
           tc.tile_pool(name="l3", bufs=3) as l3p, \
           tc.tile_pool(name="spmm_ps", bufs=4, space="PSUM") as spp, \
           tc.tile_pool(name="z_ps", bufs=2, space="PSUM") as zpp, \
           tc.tile_pool(name="t_ps", bufs=2, space="PSUM") as tpp, \
           tc.tile_pool(name="dram", bufs=1, space="DRAM") as dr:

        # ---- constants ----
        Wt = []
        for li, (din, dout) in enumerate(c.DIMS):
            w = cp.tile([P, din // P, dout], bf16, name=f"Wt{li}")
            nc.sync.dma_start(out=w[:, :, :], in_=Wd[li][:, :, :])
            Wt.append(w)
        brow = []
        for li, (din, dout) in enumerate(c.DIMS):
            b = cp.tile([1, dout], bf16, name=f"brow{li}")
            nc.sync.dma_start(out=b[:, :], in_=bd[li][:, :])
            brow.append(b)
        gb_bc = []
        for gi, gt in enumerate(gbe):
            t_ = cp.tile([P, c.D_HID], f32, name=f"gbbc{gi}")
            bcast = bass.AP(tensor=gt, offset=0, ap=[[0, P], [1, c.D_HID]])
            nc.gpsimd.dma_start(out=t_[:, :], in_=bcast)
            gb_bc.append(t_)
        eps_t = cp.tile([P, 1], f32, name="eps_t")
        nc.vector.memset(eps_t[:, :], c.EPS)
        ident = cp.tile([P, P], f32, name="ident")
        make_identity(nc, ident[:, :])
        iota_i = cp.tile([P, c.CWIN * P], i16, name="iota_i")
        nc.gpsimd.iota(iota_i[:, :], pattern=[[0, c.CWIN], [1, P]], base=0,
                       channel_multiplier=0)
        iota_b = cp.tile([P, c.CWIN, P], bf16, name="iota_b")
        nc.vector.tensor_copy(out=iota_b[:, :, :],
                              in_=iota_i[:, :].rearrange("p (w d) -> p w d",
                                                         w=c.CWIN))
        deg_c = cp.tile([P, NT], f32, name="deg_c")
        nc.sync.dma_start(out=deg_c[:, :], in_=degc_d[:, :])
        dis_c = cp.tile([P, NT], f32, name="dis_c")
        nc.scalar.activation(out=dis_c[:, :], in_=deg_c[:, :], func=AF.Sqrt)
        nc.vector.reciprocal(out=dis_c[:, :], in_=dis_c[:, :])
        deg_r = cp.tile([1, c.NPC], f32, name="deg_r")
        nc.sync.dma_start(out=deg_r[:, :], in_=degr_d[:, :])
        invdis_r = cp.tile([1, c.NPC], bf16, name="invdis_r")
        nc.scalar.activation(out=invdis_r[:, :], in_=deg_r[:, :], func=AF.Sqrt)
        batch_c = cp.tile([P, NT], bf16, name="batch_c")
        nc.sync.dma_start(out=batch_c[:, :], in_=batch_d[:, :])
        idx_t = cp.tile([P, pr.totidx // 16], i16, name="idx_t")
        nc.sync.dma_start(out=idx_t[:, :], in_=idx_d[:, :])
        dstl_t = cp.tile([P, pr.nchunks], bf16, name="dstl_t")
        nc.sync.dma_start(out=dstl_t[:, :], in_=dstl_d[:, :])

        # ---- comm buffers ----
        zsin = [[dr.tile([c.QROWS, c.DIMS[li][1]], bf16, name=f"zsin{li}_{q}",
                         tag=f"zsin{li}_{q}")
                 for q in range(NQ)] for li in range(3)]
        # NOTE: dma_gather from the Shared scratchpad region faults the device
        # (NRT_EXEC_UNIT_UNRECOVERABLE) beyond small sizes; AllGather into a
        # Local internal tile works (bass warns about perf only).
        tab_space = "Shared" if False else "Local"
        zstab = [[dr.tile([c.TABROWS, c.DIMS[li][1]], bf16,
                          name=f"zstab{li}_{q}", tag=f"zstab{li}_{q}",
                          addr_space=tab_space)
                  for q in range(NQ)] for li in range(3)]
        pool_in = dr.tile([c.NG, 132], f32, name="pool_in", tag="pool_in")
        pool_ag = dr.tile([c.NG, 132], f32, name="pool_ag", tag="pool_ag",
                          addr_space="Shared")

        spmm_ps = [None] * NT      # live PSUM tile per dst tile

        def phase_a_tile(li, t_):
            """Produce zs(layer li, one node tile) from h(li-1); li=0 reads x."""
            din, dout = c.DIMS[li]
            if li == 0:
                hrow = hrp.tile([P, c.D_HID], f32, name="hrow", tag="hrow")
                nc.sync.dma_start(out=hrow[:, :din],
                                  in_=x_in[t_ * P:(t_ + 1) * P, :])
            else:
                ps = spmm_ps[t_]
                spmm_ps[t_] = None
                hrow = hrp.tile([P, c.D_HID], f32, name="hrow", tag="hrow")
                nc.scalar.activation(out=hrow[:, :din], in_=ps[:, :din],
                                     func=AF.Relu, scale=dis_c[:, t_:t_ + 1])
                st = hrp.tile([P, 6], f32, name="bnst", tag="bnst")
                nc.vector.bn_stats(out=st[:, :], in_=hrow[:, :din])
                mv = hrp.tile([P, 2], f32, name="bnmv", tag="bnmv")
                nc.vector.bn_aggr(out=mv[:, :], in_=st[:, :])
                rs = hrp.tile([P, 1], f32, name="rstd", tag="rstd")
                nc.scalar.activation(out=rs[:, :], in_=mv[:, 1:2], func=AF.Sqrt,
                                     bias=eps_t[:, :])
                nc.vector.reciprocal(out=rs[:, :], in_=rs[:, :])
                nc.vector.tensor_scalar(out=hrow[:, :din], in0=hrow[:, :din],
                                        scalar1=mv[:, 0:1], scalar2=rs[:, :],
                                        op0=OP.subtract, op1=OP.mult)
                gi = 2 * (li - 1)
                nc.vector.tensor_tensor(out=hrow[:, :din], in0=hrow[:, :din],
                                        in1=gb_bc[gi][:, :din], op=OP.mult)
                nc.vector.tensor_tensor(out=hrow[:, :din], in0=hrow[:, :din],
                                        in1=gb_bc[gi + 1][:, :din], op=OP.add)
            tps = tpp.tile([P, din // P, P], f32, name="tps", tag="tps")
            for ih in range(din // P):
                nc.tensor.transpose(out=tps[:, ih, :],
                                    in_=hrow[:, ih * P:(ih + 1) * P],
                                    identity=ident[:, :])
            hT = htp.tile([P, din // P, P], bf16, name="hT", tag="hT")
            nc.vector.tensor_copy(out=hT[:, :, :], in_=tps[:, :, :])
            zp = zpp.tile([P, c.D_HID], f32, name="zp", tag="zp")
            for ih in range(din // P):
                nc.tensor.matmul(out=zp[:, :dout], lhsT=hT[:, ih, :],
                                 rhs=Wt[li][:, ih, :], start=(ih == 0),
                                 stop=(ih == din // P - 1),
                                 skip_group_check=True)
            zst = zsp.tile([P, c.D_HID], bf16, name="zst", tag="zst")
            nc.scalar.activation(out=zst[:, :dout], in_=zp[:, :dout],
                                 func=AF.Copy, scale=dis_c[:, t_:t_ + 1])
            q = t_ // (NT // NQ)
            r0 = (t_ % (NT // NQ)) * P
            nc.sync.dma_start(out=zsin[li][q][r0:r0 + P, :], in_=zst[:, :dout])

        def ag_maybe(li, g):
            if (g + 1) % c.GPQ == 0:
                q = g // c.GPQ
                nc.gpsimd.collective_compute(
                    "AllGather", OP.bypass, replica_groups=RG,
                    ins=[zsin[li][q][:, :].opt()], outs=[zstab[li][q][:, :].opt()])

        def phase_b_group(li, g):
            """Gather + segment-sum matmuls for dst tiles of group g, layer li."""
            dout = c.DIMS[li][1]
            g0 = int(pr.group_base[g])
            ns = int(pr.group_base[g + 1] if g + 1 < NGROUPS else pr.nchunks) - g0
            gb = gbp.tile([P, pr.smax, dout], bf16, name="gb", tag="gb")
            if c.DBG_NO_GATHER:
                nc.vector.memset(gb[:, :ns, :], 0.001)
            else:
                for q in range(NQ):
                    n_all = int(pr.call_n[g, q])
                    src_tab = zstab[li][q]
                    for o in range(0, n_all, c.MAX_CALL_IDX):
                        n_idx = min(c.MAX_CALL_IDX, n_all - o)
                        b0 = int(pr.call_base[g, q]) - g0 + o // P
                        col0 = (int(pr.call_base[g, q]) * P + o) // 16
                        nc.gpsimd.dma_gather(
                            gb[:, b0:b0 + n_idx // P, :],
                            src_tab[:, :],
                            idx_t[:, col0:col0 + n_idx // 16],
                            n_idx, n_idx, dout,
                            single_packet=c.SINGLE_PACKET)
            cw_of = {}
            for w0 in range(0, ns, c.CWIN):
                wl = min(c.CWIN, ns - w0)
                cw = cwp.tile([P, c.CWIN, P], bf16, name="cw", tag="cw")
                nc.vector.tensor_tensor(
                    out=cw[:, :wl, :],
                    in0=dstl_t[:, g0 + w0:g0 + w0 + wl, None].broadcast_to(
                        [P, wl, P]),
                    in1=iota_b[:, :wl, :], op=OP.is_equal)
                for j in range(wl):
                    cw_of[g0 + w0 + j] = (cw, j)
            for t_ in range(g * GROUP, (g + 1) * GROUP):
                ps = spp.tile([P, c.D_HID], f32, name="sps", tag="sps")
                spmm_ps[t_] = ps
                slots = pr.tile_slots[t_]
                for k, s in enumerate(slots):
                    cw, j = cw_of[s]
                    nc.tensor.matmul(out=ps[:, :dout], lhsT=cw[:, j, :],
                                     rhs=gb[:, s - g0, :], start=(k == 0),
                                     stop=False, skip_group_check=True)
                nc.tensor.matmul(out=ps[:, :dout],
                                 lhsT=invdis_r[:, t_ * P:(t_ + 1) * P],
                                 rhs=brow[li][:, :], start=False, stop=True,
                                 skip_group_check=True)

        # ---- bootstrap: phase A of layer 1 from x ----
        for g in range(NGROUPS):
            for t_ in range(g * GROUP, (g + 1) * GROUP):
                phase_a_tile(0, t_)
            ag_maybe(0, g)

        pool_ps = None
        for li in range(3):
            dout = c.DIMS[li][1]
            if li == 2:
                pool_ps = zpp.tile([c.NG, 132], f32, name="poolps", tag="zp")
            for g in range(NGROUPS):
                phase_b_group(li, g)
                if li < 2:
                    for t_ in range(g * GROUP, (g + 1) * GROUP):
                        phase_a_tile(li + 1, t_)
                    ag_maybe(li + 1, g)
                else:
                    for t_ in range(g * GROUP, (g + 1) * GROUP):
                        ps = spmm_ps[t_]
                        spmm_ps[t_] = None
                        h3 = hrp.tile([P, c.D_HID], f32, name="hrow", tag="hrow")
                        nc.scalar.activation(out=h3[:, :dout], in_=ps[:, :dout],
                                             func=AF.Copy,
                                             scale=dis_c[:, t_:t_ + 1])
                        nc.sync.dma_start(out=h_out[t_ * P:(t_ + 1) * P, :],
                                          in_=h3[:, :dout])
                        h3b = l3p.tile([P, dout + 4], bf16, name="h3b", tag="h3b")
                        nc.vector.tensor_copy(out=h3b[:, :dout], in_=h3[:, :dout])
                        nc.vector.memset(h3b[:, dout:dout + 1], 1.0)
                        oh = l3p.tile([P, c.NG], bf16, name="oh", tag="oh")
                        nc.vector.tensor_tensor(
                            out=oh[:, :],
                            in0=batch_c[:, t_:t_ + 1].broadcast_to([P, c.NG]),
                            in1=iota_b[:, 0, :c.NG], op=OP.is_equal)
                        nc.tensor.matmul(out=pool_ps[:, :dout + 1],
                                         lhsT=oh[:, :], rhs=h3b[:, :dout + 1],
                                         start=(t_ == 0), stop=(t_ == NT - 1),
                                         skip_group_check=True)

        # ---- pooled tail ----
        psb = cp.tile([c.NG, 132], f32, name="psb")
        nc.vector.memset(psb[:, :], 0.0)
        nc.vector.tensor_copy(out=psb[:, :c.D_OUT + 1],
                              in_=pool_ps[:, :c.D_OUT + 1])
        nc.sync.dma_start(out=pool_in[:, :], in_=psb[:, :])
        nc.gpsimd.collective_compute(
            "AllReduce", mybir_add(nc), replica_groups=RG,
            ins=[pool_in[:, :].opt()], outs=[pool_ag[:, :].opt()])
        pres = cp.tile([c.NG, 132], f32, name="pres")
        nc.sync.dma_start(out=pres[:, :], in_=pool_ag[:, :])
        cnt = cp.tile([c.NG, 1], f32, name="cnt")
        nc.vector.tensor_scalar(out=cnt[:, :], in0=pres[:, c.D_OUT:c.D_OUT + 1],
                                scalar1=1.0, scalar2=None, op0=OP.max)
        nc.vector.reciprocal(out=cnt[:, :], in_=cnt[:, :])
        pooled = cp.tile([c.NG, c.D_OUT], f32, name="pooled")
        nc.vector.tensor_scalar(out=pooled[:, :], in0=pres[:, :c.D_OUT],
                                scalar1=cnt[:, :], scalar2=None, op0=OP.mult)
        nc.sync.dma_start(out=pooled_out[:, :], in_=pooled[:, :])

    nc.compile()
    return nc


def mybir_add(nc):
    import concourse.mybir as mybir
    return mybir.AluOpType.add


# ---------------------------------------------------------------------------
# per-core input maps
# ---------------------------------------------------------------------------

def make_in_maps(cfg, pr, inputs):
    c = cfg
    x = np.asarray(inputs["x"], np.float32)
    batch = np.asarray(inputs["batch"], np.int64)
    Ws = [np.asarray(inputs[k], np.float32) for k in ("W1", "W2", "W3")]
    bs = [np.asarray(inputs[k], np.float32) for k in ("b1", "b2", "b3")]
    gs = [np.asarray(inputs[k], np.float32) for k in ("g1", "be1", "g2", "be2")]

    maps = []
    for ci in range(c.NCORES):
        sl = slice(ci * c.NPC, (ci + 1) * c.NPC)
        m = {"x_c": np.ascontiguousarray(x[sl])}
        for li in range(3):
            W = Ws[li]
            din, dout = c.DIMS[li]
            m[f"W{li + 1}c"] = np.ascontiguousarray(
                W.reshape(din // c.P, c.P, dout).transpose(1, 0, 2).astype(BF16))
            m[f"b{li + 1}c"] = bs[li].reshape(1, -1).astype(BF16)
        for gi, n in enumerate(("g1c", "be1c", "g2c", "be2c")):
            m[n] = np.ascontiguousarray(gs[gi])
        dg = pr.deg[sl]
        m["deg_col"] = np.ascontiguousarray(dg.reshape(c.NT, c.P).T)
        m["deg_row"] = np.ascontiguousarray(dg.reshape(1, c.NPC))
        m["batch_col"] = np.ascontiguousarray(
            batch[sl].reshape(c.NT, c.P).T.astype(BF16))
        m["idx16"] = np.ascontiguousarray(pr.idx16[ci])
        m["dstloc"] = np.ascontiguousarray(pr.dstloc[ci])
        maps.append(m)
    return maps


# ---------------------------------------------------------------------------
# entry points
# ---------------------------------------------------------------------------

_CACHE = {}


def _build(inputs, cfg=None):
    cfg = cfg or Cfg()
    key = (cfg.N, cfg.E, cfg.NG, cfg.NQ, cfg.GROUP)
    if key not in _CACHE:
        pr = prep_edges(cfg, np.asarray(inputs["edge_index"], np.int64),
                        np.asarray(inputs["batch"], np.int64))
        nc = build_program(cfg, pr)
        _CACHE[key] = (cfg, pr, nc)
    cfg, pr, nc = _CACHE[key]
    in_maps = make_in_maps(cfg, pr, inputs)
    return cfg, pr, nc, in_maps


def _run(inputs, trace=False, cfg=None):
    from concourse import bass_utils
    cfg, pr, nc, in_maps = _build(inputs, cfg)
    res = bass_utils.run_bass_kernel_spmd(
        nc, in_maps, core_ids=list(range(cfg.NCORES)), trace=False)
    h = np.concatenate([r["h_out"] for r in res.results], axis=0)
    pooled = res.results[0]["pooled_out"]
    return (h.astype(np.float32), pooled.astype(np.float32)), res


def bench(inputs, iters=10, cfg=None):
    """Build the sharded PJRT executable once, keep inputs device-resident,
    and wall-clock repeated executions. Returns ((h, pooled), best_ns)."""
    import time

    import jax
    from jax.sharding import Mesh, NamedSharding, PartitionSpec
    try:
        from jax.experimental.shard_map import shard_map
    except ImportError:
        from jax.sharding import shard_map
    import concourse.mybir as mybir
    from concourse import bass2jax

    cfg, pr, nc, in_maps = _build(inputs, cfg)
    n_cores = cfg.NCORES
    bass2jax.install_neuronx_cc_hook()

    partition_name = (nc.partition_id_tensor.name
                      if nc.partition_id_tensor else None)
    in_names, out_names, out_avals, zero_outs = [], [], [], []
    for alloc in nc.m.functions[0].allocations:
        if not isinstance(alloc, mybir.MemoryLocationSet):
            continue
        name = alloc.memorylocations[0].name
        if alloc.kind == "ExternalInput":
            if name != partition_name:
                in_names.append(name)
        elif alloc.kind == "ExternalOutput":
            shape = tuple(alloc.tensor_shape)
            dtype = mybir.dt.np(alloc.dtype)
            out_names.append(name)
            out_avals.append(jax.core.ShapedArray(shape, dtype))
            zero_outs.append(np.zeros(shape, dtype))
    n_params = len(in_names)
    all_in_names = in_names + out_names
    if partition_name is not None:
        all_in_names = all_in_names + [partition_name]

    def _exec_once(ins, zeros):
        operands = list(ins) + list(zeros)
        if partition_name is not None:
            operands.append(bass2jax.partition_id_tensor())
        outs = bass2jax._bass_exec_p.bind(
            *operands,
            out_avals=tuple(out_avals),
            in_names=tuple(all_in_names),
            out_names=tuple(out_names),
            lowering_input_output_aliases=(),
            sim_require_finite=True,
            sim_require_nnan=True,
            nc=nc,
        )
        return tuple(outs)

    chain = int(os.environ.get("BENCH_CHAIN", "1"))

    def _body(*args):
        ins, zeros = args[:n_params], args[n_params:]
        outs = _exec_once(ins, zeros)
        for _ in range(chain - 1):
            outs = _exec_once(ins, outs)
        return outs

    devices = jax.devices()[:n_cores]
    mesh = Mesh(np.asarray(devices), ("core",))
    spec = PartitionSpec("core")
    sharded = jax.jit(
        shard_map(_body, mesh=mesh, in_specs=(spec,) * (n_params + len(out_names)),
                  out_specs=(spec,) * len(out_names), check_rep=False),
        keep_unused=True)
    sh = NamedSharding(mesh, spec)
    concat_in = [
        jax.device_put(
            np.concatenate([np.asarray(in_maps[c][n]) for c in range(n_cores)],
                           axis=0), sh)
        for n in in_names
    ]
    concat_zeros = [
        jax.device_put(np.zeros((n_cores * z.shape[0], *z.shape[1:]), z.dtype), sh)
        for z in zero_outs
    ]
    out = sharded(*concat_in, *concat_zeros)
    jax.block_until_ready(out)
    times = []
    for _ in range(iters):
        t0 = time.perf_counter()
        out = sharded(*concat_in, *concat_zeros)
        jax.block_until_ready(out)
        times.append(time.perf_counter() - t0)
    best_ns = int(min(times) * 1e9)
    outs = {n: np.asarray(out[i]) for i, n in enumerate(out_names)}
    h = outs["h_out"].reshape(n_cores, cfg.NPC, cfg.D_OUT).reshape(-1, cfg.D_OUT)
    pooled = outs["pooled_out"].reshape(n_cores, cfg.NG, cfg.D_OUT)[0]
    print("bench times (ms):", [f"{t * 1e3:.3f}" for t in times])
    return (h.astype(np.float32), pooled.astype(np.float32)), best_ns


def kernel(**inputs):
    (h, pooled), _ = _run(inputs, trace=False)
    return h, pooled


# revision 16
# speedup vs baseline: 107.6945x; 78.5116x over previous
"""GCN encoder (3-layer message passing + LayerNorm + mean pool) on 8 trn2 cores.

Strategy (see also spec sharding_hint):
  - Nodes partitioned 8 ways by id (dst-sharding). Self loops appended as
    ordinary edges. Per core, edges grouped by (dst-tile of 128, src-quarter)
    with a *shared* padded structure (max over cores) so a single SPMD NEFF
    serves all cores; per-core data = int16 gather indices + local-dst ids.
  - Per layer: dense z = h @ W on PE (activations PE-transposed on the fly),
    z scaled by deg^-1/2, cast bf16, AllGather'd in 4 node-quarter chunks
    (tables of 16384 rows keep dma_gather's int16 indices in range and let
    gathers of quarter q overlap the AllGather of quarter q+1).
  - Message passing: per-edge dma_gather of 512B rows into SBUF, segment-sum
    via one-hot(dst_local) matmuls accumulating in PSUM; bias added with a
    rank-1 (sqrt(deg) x b) matmul; epilogue relu+LayerNorm on ACT/DVE.
  - Mean pool: one-hot(batch) matmuls with an appended ones column, 33KB
    AllReduce, divide by max(count, 1).
"""

import math
import os
import sys

import numpy as np

for p in ("/opt/trn_rl_repo",):
    if p not in sys.path and os.path.isdir(p):
        sys.path.insert(0, p)

import ml_dtypes

BF16 = ml_dtypes.bfloat16


# ---------------------------------------------------------------------------
# configuration
# ---------------------------------------------------------------------------

class Cfg:
    def __init__(self, N=65536, E=1048576, NG=64, D_IN=256, D_HID=256, D_OUT=128,
                 NQ=2, GROUP=4, EPS=1e-5):
        self.N, self.E, self.NG = N, E, NG
        self.D_IN, self.D_HID, self.D_OUT = D_IN, D_HID, D_OUT
        self.EPS = EPS
        self.NCORES = 8
        self.P = 128
        self.NPC = N // self.NCORES                  # nodes per core
        self.NT = self.NPC // self.P                 # dst tiles per core
        self.NQ = NQ                                 # AllGather chunks (src quarters)
        self.QROWS = self.NPC // NQ                  # local rows per quarter
        self.TABROWS = self.QROWS * self.NCORES      # gather-table rows (int16 bound)
        assert self.TABROWS <= 32768
        self.GROUP = GROUP                           # dst tiles per gather group
        self.NGROUPS = self.NT // GROUP
        self.GPQ = self.NGROUPS // NQ                # groups per quarter
        assert self.NT % GROUP == 0 and self.NGROUPS % NQ == 0
        assert self.QROWS % self.P == 0
        self.DIMS = [(D_IN, D_HID), (D_HID, D_HID), (D_HID, D_OUT)]
        self.CWIN = 8                                # chunks per one-hot build
        self.DBG_NO_GATHER = False                   # crash bisection knobs
        self.DBG_LOCAL_TABLE = False
        self.MAX_CALL_IDX = 1 << 30                  # split big dma_gather calls
        self.SINGLE_PACKET = False  # single_packet=True serializes + faults >2k idx
        self.GBUFS = 2


# ---------------------------------------------------------------------------
# host-side edge preprocessing (pure integer index work)
# ---------------------------------------------------------------------------

class Prep:
    pass


def prep_edges(cfg, edge_index, batch):
    """Sort/partition edges, build the shared padded segment structure and the
    per-core int16 gather-index / local-dst arrays."""
    c = cfg
    P, NPC, NT, NQ, QROWS, GROUP = c.P, c.NPC, c.NT, c.NQ, c.QROWS, c.GROUP
    src = np.concatenate([np.asarray(edge_index[0], np.int64),
                          np.arange(c.N, dtype=np.int64)])
    dst = np.concatenate([np.asarray(edge_index[1], np.int64),
                          np.arange(c.N, dtype=np.int64)])
    deg = np.bincount(dst, minlength=c.N).astype(np.float32)

    core = dst // NPC
    q_of = (src % NPC) // QROWS                       # AllGather chunk of src
    gidx = (src // NPC) * QROWS + (src % NPC) - q_of * QROWS  # idx in table q

    per_core = []
    counts = np.zeros((c.NCORES, NT, NQ), np.int64)
    for ci in range(c.NCORES):
        m = core == ci
        s_q, s_g, d_l = q_of[m], gidx[m], dst[m] - ci * NPC
        t = d_l // P
        grp = t // GROUP
        order = np.lexsort((t, s_q, grp))             # group -> quarter -> tile
        s_q, s_g, d_l, t = s_q[order], s_g[order], d_l[order], t[order]
        cnt = np.bincount(t * NQ + s_q, minlength=NT * NQ).reshape(NT, NQ)
        counts[ci] = cnt
        per_core.append((s_q, s_g, d_l % P, t))

    pad = ((counts.max(axis=0) + P - 1) // P) * P     # shared [NT, NQ] padded sizes

    # stream layout: for g: for q: for t in g: seg(t, q)
    seg_base = np.zeros((NT, NQ), np.int64)           # slot offset of each segment
    call_base = np.zeros((c.NGROUPS, NQ), np.int64)   # slot offset of each gather call
    call_n = np.zeros((c.NGROUPS, NQ), np.int64)      # num_idxs per call
    group_base = np.zeros(c.NGROUPS, np.int64)
    pos = 0                                           # in slots (chunks of 128)
    for g in range(c.NGROUPS):
        group_base[g] = pos
        for q in range(NQ):
            call_base[g, q] = pos
            for t in range(g * GROUP, (g + 1) * GROUP):
                seg_base[t, q] = pos
                pos += pad[t, q] // P
            call_n[g, q] = (pos - call_base[g, q]) * P
    nchunks = pos
    totidx = nchunks * P

    pr = Prep()
    pr.deg = deg
    pr.pad, pr.seg_base = pad, seg_base
    pr.call_base, pr.call_n, pr.group_base = call_base, call_n, group_base
    pr.nchunks, pr.totidx = nchunks, totidx
    pr.smax = int((np.diff(np.append(group_base, nchunks))).max())
    # ordered chunk list per tile: global slot ids
    pr.tile_slots = [
        [int(s) for q in range(NQ)
         for s in range(seg_base[t, q], seg_base[t, q] + pad[t, q] // P)]
        for t in range(NT)
    ]

    pr.idx16, pr.dstloc = [], []
    for ci in range(c.NCORES):
        s_q, s_g, dl, t = per_core[ci]
        idx = np.zeros(totidx, np.int16)
        dloc = np.full(totidx, -1.0, np.float32)
        # per-(t,q) segment fill
        start = np.zeros((NT, NQ), np.int64)
        start[:, :] = seg_base * P
        # edges are already in (g, q, t) order; compute per-seg offsets
        key = t * NQ + s_q
        # stable positions within each segment
        seg_begin = (seg_base * P)[t, s_q]
        # rank within segment: since sorted by (grp, q, t), same-(t,q) edges
        # are contiguous; use cumulative count trick
        _, first_pos, inv = np.unique(key, return_index=True, return_inverse=True)
        offs = np.arange(len(key)) - first_pos[inv]
        ppos = seg_begin + offs
        idx[ppos] = s_g.astype(np.int16)
        dloc[ppos] = dl.astype(np.float32)
        cols = totidx // 16
        pr.idx16.append(np.tile(idx.reshape(cols, 16).T, (8, 1)))   # [128, cols]
        pr.dstloc.append(np.ascontiguousarray(
            dloc.reshape(nchunks, P).T.astype(BF16)))               # [128, nchunks]
    return pr


# ---------------------------------------------------------------------------
# bass program
# ---------------------------------------------------------------------------

def build_program(cfg, pr):
    import concourse.bass as bass
    import concourse.mybir as mybir
    import concourse.tile as tile
    from concourse import bacc
    from concourse.masks import make_identity
    from concourse._compat import axon_active

    c = cfg
    P, NT, NQ, NGROUPS, GROUP = c.P, c.NT, c.NQ, c.NGROUPS, c.GROUP
    f32, bf16, i16 = mybir.dt.float32, mybir.dt.bfloat16, mybir.dt.int16
    AF = mybir.ActivationFunctionType
    OP = mybir.AluOpType
    RG = [list(range(c.NCORES))]

    nc = bacc.Bacc("TRN2", target_bir_lowering=False, debug=False,
                   num_devices=c.NCORES)

    # ---- I/O ----
    x_in = nc.dram_tensor("x_c", [c.NPC, c.D_IN], f32, kind="ExternalInput")
    Wd, bd = [], []
    for li, (din, dout) in enumerate(c.DIMS):
        Wd.append(nc.dram_tensor(f"W{li + 1}c", [P, din // P, dout], bf16,
                                 kind="ExternalInput"))
        bd.append(nc.dram_tensor(f"b{li + 1}c", [1, dout], bf16,
                                 kind="ExternalInput"))
    gbe = [nc.dram_tensor(n, [c.D_HID], f32, kind="ExternalInput")
           for n in ("g1c", "be1c", "g2c", "be2c")]
    degc_d = nc.dram_tensor("deg_col", [P, NT], f32, kind="ExternalInput")
    degr_d = nc.dram_tensor("deg_row", [1, c.NPC], f32, kind="ExternalInput")
    batch_d = nc.dram_tensor("batch_col", [P, NT], bf16, kind="ExternalInput")
    idx_d = nc.dram_tensor("idx16", [P, pr.totidx // 16], i16, kind="ExternalInput")
    dstl_d = nc.dram_tensor("dstloc", [P, pr.nchunks], bf16, kind="ExternalInput")
    h_out = nc.dram_tensor("h_out", [c.NPC, c.D_OUT], f32, kind="ExternalOutput")
    pooled_out = nc.dram_tensor("pooled_out", [c.NG, c.D_OUT], f32,
                                kind="ExternalOutput")

    with tile.TileContext(nc, num_cores=c.NCORES) as tc:
      with tc.tile_pool(name="const", bufs=1) as cp, \
           tc.tile_pool(name="hrow", bufs=3) as hrp, \
           tc.tile_pool(name="ht", bufs=3) as htp, \
           tc.tile_pool(name="zs", bufs=4) as zsp, \
           tc.tile_pool(name="gbuf", bufs=c.GBUFS) as gbp, \
           tc.tile_pool(name="cwin", bufs=4) as cwp, \
           tc.tile_pool(name="l3", bufs=3) as l3p, \
           tc.tile_pool(name="spmm_ps", bufs=4, space="PSUM") as spp, \
           tc.tile_pool(name="z_ps", bufs=2, space="PSUM") as zpp, \
           tc.tile_pool(name="t_ps", bufs=2, space="PSUM") as tpp, \
           tc.tile_pool(name="dram", bufs=1, space="DRAM") as dr:

        # ---- constants ----
        Wt = []
        for li, (din, dout) in enumerate(c.DIMS):
            w = cp.tile([P, din // P, dout], bf16, name=f"Wt{li}")
            nc.sync.dma_start(out=w[:, :, :], in_=Wd[li][:, :, :])
            Wt.append(w)
        brow = []
        for li, (din, dout) in enumerate(c.DIMS):
            b = cp.tile([1, dout], bf16, name=f"brow{li}")
            nc.sync.dma_start(out=b[:, :], in_=bd[li][:, :])
            brow.append(b)
        gb_bc = []
        for gi, gt in enumerate(gbe):
            t_ = cp.tile([P, c.D_HID], f32, name=f"gbbc{gi}")
            bcast = bass.AP(tensor=gt, offset=0, ap=[[0, P], [1, c.D_HID]])
            nc.gpsimd.dma_start(out=t_[:, :], in_=bcast)
            gb_bc.append(t_)
        eps_t = cp.tile([P, 1], f32, name="eps_t")
        nc.vector.memset(eps_t[:, :], c.EPS)
        ident = cp.tile([P, P], f32, name="ident")
        make_identity(nc, ident[:, :])
        iota_i = cp.tile([P, c.CWIN * P], i16, name="iota_i")
        nc.gpsimd.iota(iota_i[:, :], pattern=[[0, c.CWIN], [1, P]], base=0,
                       channel_multiplier=0)
        iota_b = cp.tile([P, c.CWIN, P], bf16, name="iota_b")
        nc.vector.tensor_copy(out=iota_b[:, :, :],
                              in_=iota_i[:, :].rearrange("p (w d) -> p w d",
                                                         w=c.CWIN))
        deg_c = cp.tile([P, NT], f32, name="deg_c")
        nc.sync.dma_start(out=deg_c[:, :], in_=degc_d[:, :])
        dis_c = cp.tile([P, NT], f32, name="dis_c")
        nc.scalar.activation(out=dis_c[:, :], in_=deg_c[:, :], func=AF.Sqrt)
        nc.vector.reciprocal(out=dis_c[:, :], in_=dis_c[:, :])
        deg_r = cp.tile([1, c.NPC], f32, name="deg_r")
        nc.sync.dma_start(out=deg_r[:, :], in_=degr_d[:, :])
        invdis_r = cp.tile([1, c.NPC], bf16, name="invdis_r")
        nc.scalar.activation(out=invdis_r[:, :], in_=deg_r[:, :], func=AF.Sqrt)
        batch_c = cp.tile([P, NT], bf16, name="batch_c")
        nc.sync.dma_start(out=batch_c[:, :], in_=batch_d[:, :])
        idx_t = cp.tile([P, pr.totidx // 16], i16, name="idx_t")
        nc.sync.dma_start(out=idx_t[:, :], in_=idx_d[:, :])
        dstl_t = cp.tile([P, pr.nchunks], bf16, name="dstl_t")
        nc.sync.dma_start(out=dstl_t[:, :], in_=dstl_d[:, :])

        # ---- comm buffers ----
        zsin = [[dr.tile([c.QROWS, c.DIMS[li][1]], bf16, name=f"zsin{li}_{q}",
                         tag=f"zsin{li}_{q}")
                 for q in range(NQ)] for li in range(3)]
        # NOTE: dma_gather from the Shared scratchpad region faults the device
        # (NRT_EXEC_UNIT_UNRECOVERABLE) beyond small sizes; AllGather into a
        # Local internal tile works (bass warns about perf only).
        tab_space = "Shared" if False else "Local"
        zstab = [[dr.tile([c.TABROWS, c.DIMS[li][1]], bf16,
                          name=f"zstab{li}_{q}", tag=f"zstab{li}_{q}",
                          addr_space=tab_space)
                  for q in range(NQ)] for li in range(3)]
        pool_in = dr.tile([c.NG, 132], f32, name="pool_in", tag="pool_in")
        pool_ag = dr.tile([c.NG, 132], f32, name="pool_ag", tag="pool_ag",
                          addr_space="Shared")

        spmm_ps = [None] * NT      # live PSUM tile per dst tile

        def phase_a_tile(li, t_):
            """Produce zs(layer li, one node tile) from h(li-1); li=0 reads x."""
            din, dout = c.DIMS[li]
            if li == 0:
                hrow = hrp.tile([P, c.D_HID], f32, name="hrow", tag="hrow")
                nc.sync.dma_start(out=hrow[:, :din],
                                  in_=x_in[t_ * P:(t_ + 1) * P, :])
            else:
                ps = spmm_ps[t_]
                spmm_ps[t_] = None
                hrow = hrp.tile([P, c.D_HID], f32, name="hrow", tag="hrow")
                nc.scalar.activation(out=hrow[:, :din], in_=ps[:, :din],
                                     func=AF.Relu, scale=dis_c[:, t_:t_ + 1])
                st = hrp.tile([P, 6], f32, name="bnst", tag="bnst")
                nc.vector.bn_stats(out=st[:, :], in_=hrow[:, :din])
                mv = hrp.tile([P, 2], f32, name="bnmv", tag="bnmv")
                nc.vector.bn_aggr(out=mv[:, :], in_=st[:, :])
                rs = hrp.tile([P, 1], f32, name="rstd", tag="rstd")
                nc.scalar.activation(out=rs[:, :], in_=mv[:, 1:2], func=AF.Sqrt,
                                     bias=eps_t[:, :])
                nc.vector.reciprocal(out=rs[:, :], in_=rs[:, :])
                nc.vector.tensor_scalar(out=hrow[:, :din], in0=hrow[:, :din],
                                        scalar1=mv[:, 0:1], scalar2=rs[:, :],
                                        op0=OP.subtract, op1=OP.mult)
                gi = 2 * (li - 1)
                nc.vector.tensor_tensor(out=hrow[:, :din], in0=hrow[:, :din],
                                        in1=gb_bc[gi][:, :din], op=OP.mult)
                nc.vector.tensor_tensor(out=hrow[:, :din], in0=hrow[:, :din],
                                        in1=gb_bc[gi + 1][:, :din], op=OP.add)
            tps = tpp.tile([P, din // P, P], f32, name="tps", tag="tps")
            for ih in range(din // P):
                nc.tensor.transpose(out=tps[:, ih, :],
                                    in_=hrow[:, ih * P:(ih + 1) * P],
                                    identity=ident[:, :])
            hT = htp.tile([P, din // P, P], bf16, name="hT", tag="hT")
            nc.vector.tensor_copy(out=hT[:, :, :], in_=tps[:, :, :])
            zp = zpp.tile([P, c.D_HID], f32, name="zp", tag="zp")
            for ih in range(din // P):
                nc.tensor.matmul(out=zp[:, :dout], lhsT=hT[:, ih, :],
                                 rhs=Wt[li][:, ih, :], start=(ih == 0),
                                 stop=(ih == din // P - 1),
                                 skip_group_check=True)
            zst = zsp.tile([P, c.D_HID], bf16, name="zst", tag="zst")
            nc.scalar.activation(out=zst[:, :dout], in_=zp[:, :dout],
                                 func=AF.Copy, scale=dis_c[:, t_:t_ + 1])
            q = t_ // (NT // NQ)
            r0 = (t_ % (NT // NQ)) * P
            nc.sync.dma_start(out=zsin[li][q][r0:r0 + P, :], in_=zst[:, :dout])

        def ag_maybe(li, g):
            if (g + 1) % c.GPQ == 0:
                q = g // c.GPQ
                nc.gpsimd.collective_compute(
                    "AllGather", OP.bypass, replica_groups=RG,
                    ins=[zsin[li][q][:, :].opt()], outs=[zstab[li][q][:, :].opt()])

        def phase_b_group(li, g):
            """Gather + segment-sum matmuls for dst tiles of group g, layer li."""
            dout = c.DIMS[li][1]
            g0 = int(pr.group_base[g])
            ns = int(pr.group_base[g + 1] if g + 1 < NGROUPS else pr.nchunks) - g0
            gb = gbp.tile([P, pr.smax, dout], bf16, name="gb", tag="gb")
            if c.DBG_NO_GATHER:
                nc.vector.memset(gb[:, :ns, :], 0.001)
            else:
                for q in range(NQ):
                    n_all = int(pr.call_n[g, q])
                    src_tab = zstab[li][q]
                    for o in range(0, n_all, c.MAX_CALL_IDX):
                        n_idx = min(c.MAX_CALL_IDX, n_all - o)
                        b0 = int(pr.call_base[g, q]) - g0 + o // P
                        col0 = (int(pr.call_base[g, q]) * P + o) // 16
                        nc.gpsimd.dma_gather(
                            gb[:, b0:b0 + n_idx // P, :],
                            src_tab[:, :],
                            idx_t[:, col0:col0 + n_idx // 16],
                            n_idx, n_idx, dout,
                            single_packet=c.SINGLE_PACKET)
            cw_of = {}
            for w0 in range(0, ns, c.CWIN):
                wl = min(c.CWIN, ns - w0)
                cw = cwp.tile([P, c.CWIN, P], bf16, name="cw", tag="cw")
                nc.vector.tensor_tensor(
                    out=cw[:, :wl, :],
                    in0=dstl_t[:, g0 + w0:g0 + w0 + wl, None].broadcast_to(
                        [P, wl, P]),
                    in1=iota_b[:, :wl, :], op=OP.is_equal)
                for j in range(wl):
                    cw_of[g0 + w0 + j] = (cw, j)
            for t_ in range(g * GROUP, (g + 1) * GROUP):
                ps = spp.tile([P, c.D_HID], f32, name="sps", tag="sps")
                spmm_ps[t_] = ps
                slots = pr.tile_slots[t_]
                for k, s in enumerate(slots):
                    cw, j = cw_of[s]
                    nc.tensor.matmul(out=ps[:, :dout], lhsT=cw[:, j, :],
                                     rhs=gb[:, s - g0, :], start=(k == 0),
                                     stop=False, skip_group_check=True)
                nc.tensor.matmul(out=ps[:, :dout],
                                 lhsT=invdis_r[:, t_ * P:(t_ + 1) * P],
                                 rhs=brow[li][:, :], start=False, stop=True,
                                 skip_group_check=True)

        # ---- bootstrap: phase A of layer 1 from x ----
        for g in range(NGROUPS):
            for t_ in range(g * GROUP, (g + 1) * GROUP):
                phase_a_tile(0, t_)
            ag_maybe(0, g)

        pool_ps = None
        for li in range(3):
            dout = c.DIMS[li][1]
            if li == 2:
                pool_ps = zpp.tile([c.NG, 132], f32, name="poolps", tag="zp")
            for g in range(NGROUPS):
                phase_b_group(li, g)
                if li < 2:
                    for t_ in range(g * GROUP, (g + 1) * GROUP):
                        phase_a_tile(li + 1, t_)
                    ag_maybe(li + 1, g)
                else:
                    for t_ in range(g * GROUP, (g + 1) * GROUP):
                        ps = spmm_ps[t_]
                        spmm_ps[t_] = None
                        h3 = hrp.tile([P, c.D_HID], f32, name="hrow", tag="hrow")
                        nc.scalar.activation(out=h3[:, :dout], in_=ps[:, :dout],
                                             func=AF.Copy,
                                             scale=dis_c[:, t_:t_ + 1])
                        nc.sync.dma_start(out=h_out[t_ * P:(t_ + 1) * P, :],
                                          in_=h3[:, :dout])
                        h3b = l3p.tile([P, dout + 4], bf16, name="h3b", tag="h3b")
                        nc.vector.tensor_copy(out=h3b[:, :dout], in_=h3[:, :dout])
                        nc.vector.memset(h3b[:, dout:dout + 1], 1.0)
                        oh = l3p.tile([P, c.NG], bf16, name="oh", tag="oh")
                        nc.vector.tensor_tensor(
                            out=oh[:, :],
                            in0=batch_c[:, t_:t_ + 1].broadcast_to([P, c.NG]),
                            in1=iota_b[:, 0, :c.NG], op=OP.is_equal)
                        nc.tensor.matmul(out=pool_ps[:, :dout + 1],
                                         lhsT=oh[:, :], rhs=h3b[:, :dout + 1],
                                         start=(t_ == 0), stop=(t_ == NT - 1),
                                         skip_group_check=True)

        # ---- pooled tail ----
        psb = cp.tile([c.NG, 132], f32, name="psb")
        nc.vector.memset(psb[:, :], 0.0)
        nc.vector.tensor_copy(out=psb[:, :c.D_OUT + 1],
                              in_=pool_ps[:, :c.D_OUT + 1])
        nc.sync.dma_start(out=pool_in[:, :], in_=psb[:, :])
        nc.gpsimd.collective_compute(
            "AllReduce", mybir_add(nc), replica_groups=RG,
            ins=[pool_in[:, :].opt()], outs=[pool_ag[:, :].opt()])
        pres = cp.tile([c.NG, 132], f32, name="pres")
        nc.sync.dma_start(out=pres[:, :], in_=pool_ag[:, :])
        cnt = cp.tile([c.NG, 1], f32, name="cnt")
        nc.vector.tensor_scalar(out=cnt[:, :], in0=pres[:, c.D_OUT:c.D_OUT + 1],
                                scalar1=1.0, scalar2=None, op0=OP.max)
        nc.vector.reciprocal(out=cnt[:, :], in_=cnt[:, :])
        pooled = cp.tile([c.NG, c.D_OUT], f32, name="pooled")
        nc.vector.tensor_scalar(out=pooled[:, :], in0=pres[:, :c.D_OUT],
                                scalar1=cnt[:, :], scalar2=None, op0=OP.mult)
        nc.sync.dma_start(out=pooled_out[:, :], in_=pooled[:, :])

    nc.compile()
    return nc


def mybir_add(nc):
    import concourse.mybir as mybir
    return mybir.AluOpType.add


# ---------------------------------------------------------------------------
# per-core input maps
# ---------------------------------------------------------------------------

def make_in_maps(cfg, pr, inputs):
    c = cfg
    x = np.asarray(inputs["x"], np.float32)
    batch = np.asarray(inputs["batch"], np.int64)
    Ws = [np.asarray(inputs[k], np.float32) for k in ("W1", "W2", "W3")]
    bs = [np.asarray(inputs[k], np.float32) for k in ("b1", "b2", "b3")]
    gs = [np.asarray(inputs[k], np.float32) for k in ("g1", "be1", "g2", "be2")]

    maps = []
    for ci in range(c.NCORES):
        sl = slice(ci * c.NPC, (ci + 1) * c.NPC)
        m = {"x_c": np.ascontiguousarray(x[sl])}
        for li in range(3):
            W = Ws[li]
            din, dout = c.DIMS[li]
            m[f"W{li + 1}c"] = np.ascontiguousarray(
                W.reshape(din // c.P, c.P, dout).transpose(1, 0, 2).astype(BF16))
            m[f"b{li + 1}c"] = bs[li].reshape(1, -1).astype(BF16)
        for gi, n in enumerate(("g1c", "be1c", "g2c", "be2c")):
            m[n] = np.ascontiguousarray(gs[gi])
        dg = pr.deg[sl]
        m["deg_col"] = np.ascontiguousarray(dg.reshape(c.NT, c.P).T)
        m["deg_row"] = np.ascontiguousarray(dg.reshape(1, c.NPC))
        m["batch_col"] = np.ascontiguousarray(
            batch[sl].reshape(c.NT, c.P).T.astype(BF16))
        m["idx16"] = np.ascontiguousarray(pr.idx16[ci])
        m["dstloc"] = np.ascontiguousarray(pr.dstloc[ci])
        maps.append(m)
    return maps


# ---------------------------------------------------------------------------
# entry points
# ---------------------------------------------------------------------------

_CACHE = {}


def _build(inputs, cfg=None):
    cfg = cfg or Cfg()
    key = (cfg.N, cfg.E, cfg.NG, cfg.NQ, cfg.GROUP)
    if key not in _CACHE:
        pr = prep_edges(cfg, np.asarray(inputs["edge_index"], np.int64),
                        np.asarray(inputs["batch"], np.int64))
        nc = build_program(cfg, pr)
        _CACHE[key] = (cfg, pr, nc)
    cfg, pr, nc = _CACHE[key]
    in_maps = make_in_maps(cfg, pr, inputs)
    return cfg, pr, nc, in_maps


def _run(inputs, trace=False, cfg=None):
    from concourse import bass_utils
    cfg, pr, nc, in_maps = _build(inputs, cfg)
    res = bass_utils.run_bass_kernel_spmd(
        nc, in_maps, core_ids=list(range(cfg.NCORES)), trace=False)
    h = np.concatenate([r["h_out"] for r in res.results], axis=0)
    pooled = res.results[0]["pooled_out"]
    return (h.astype(np.float32), pooled.astype(np.float32)), res


def bench(inputs, iters=10, cfg=None):
    """Build the sharded PJRT executable once, keep inputs device-resident,
    and wall-clock repeated executions. Returns ((h, pooled), best_ns)."""
    import time

    import jax
    from jax.sharding import Mesh, NamedSharding, PartitionSpec
    try:
        from jax.experimental.shard_map import shard_map
    except ImportError:
        from jax.sharding import shard_map
    import concourse.mybir as mybir
    from concourse import bass2jax

    cfg, pr, nc, in_maps = _build(inputs, cfg)
    n_cores = cfg.NCORES
    bass2jax.install_neuronx_cc_hook()

    partition_name = (nc.partition_id_tensor.name
                      if nc.partition_id_tensor else None)
    in_names, out_names, out_avals, zero_outs = [], [], [], []
    for alloc in nc.m.functions[0].allocations:
        if not isinstance(alloc, mybir.MemoryLocationSet):
            continue
        name = alloc.memorylocations[0].name
        if alloc.kind == "ExternalInput":
            if name != partition_name:
                in_names.append(name)
        elif alloc.kind == "ExternalOutput":
            shape = tuple(alloc.tensor_shape)
            dtype = mybir.dt.np(alloc.dtype)
            out_names.append(name)
            out_avals.append(jax.core.ShapedArray(shape, dtype))
            zero_outs.append(np.zeros(shape, dtype))
    n_params = len(in_names)
    all_in_names = in_names + out_names
    if partition_name is not None:
        all_in_names = all_in_names + [partition_name]

    def _exec_once(ins, zeros):
        operands = list(ins) + list(zeros)
        if partition_name is not None:
            operands.append(bass2jax.partition_id_tensor())
        outs = bass2jax._bass_exec_p.bind(
            *operands,
            out_avals=tuple(out_avals),
            in_names=tuple(all_in_names),
            out_names=tuple(out_names),
            lowering_input_output_aliases=(),
            sim_require_finite=True,
            sim_require_nnan=True,
            nc=nc,
        )
        return tuple(outs)

    chain = int(os.environ.get("BENCH_CHAIN", "1"))

    def _body(*args):
        ins, zeros = args[:n_params], args[n_params:]
        outs = _exec_once(ins, zeros)
        for _ in range(chain - 1):
            outs = _exec_once(ins, outs)
        return outs

    devices = jax.devices()[:n_cores]
    mesh = Mesh(np.asarray(devices), ("core",))
    spec = PartitionSpec("core")
    sharded = jax.jit(
        shard_map(_body, mesh=mesh, in_specs=(spec,) * (n_params + len(out_names)),
                  out_specs=(spec,) * len(out_names), check_rep=False),
        keep_unused=True)
    sh = NamedSharding(mesh, spec)
    concat_in = [
        jax.device_put(
            np.concatenate([np.asarray(in_maps[c][n]) for c in range(n_cores)],
                           axis=0), sh)
        for n in in_names
    ]
    concat_zeros = [
        jax.device_put(np.zeros((n_cores * z.shape[0], *z.shape[1:]), z.dtype), sh)
        for z in zero_outs
    ]
    out = sharded(*concat_in, *concat_zeros)
    jax.block_until_ready(out)
    times = []
    for _ in range(iters):
        t0 = time.perf_counter()
        out = sharded(*concat_in, *concat_zeros)
        jax.block_until_ready(out)
        times.append(time.perf_counter() - t0)
    best_ns = int(min(times) * 1e9)
    outs = {n: np.asarray(out[i]) for i, n in enumerate(out_names)}
    h = outs["h_out"].reshape(n_cores, cfg.NPC, cfg.D_OUT).reshape(-1, cfg.D_OUT)
    pooled = outs["pooled_out"].reshape(n_cores, cfg.NG, cfg.D_OUT)[0]
    print("bench times (ms):", [f"{t * 1e3:.3f}" for t in times])
    return (h.astype(np.float32), pooled.astype(np.float32)), best_ns


def kernel(**inputs):
    (h, pooled), _ = _run(inputs, trace=False)
    return h, pooled


# revision 21
# speedup vs baseline: 325.8185x; 3.0254x over previous
"""GCN encoder (3-layer message passing + LayerNorm + mean pool) on 8 trn2 cores.

Strategy (see also spec sharding_hint):
  - Nodes partitioned 8 ways by id (dst-sharding). Self loops appended as
    ordinary edges. Per core, edges grouped by (dst-tile of 128, src-quarter)
    with a *shared* padded structure (max over cores) so a single SPMD NEFF
    serves all cores; per-core data = int16 gather indices + local-dst ids.
  - Per layer: dense z = h @ W on PE (activations PE-transposed on the fly),
    z scaled by deg^-1/2, cast bf16, AllGather'd in 4 node-quarter chunks
    (tables of 16384 rows keep dma_gather's int16 indices in range and let
    gathers of quarter q overlap the AllGather of quarter q+1).
  - Message passing: per-edge dma_gather of 512B rows into SBUF, segment-sum
    via one-hot(dst_local) matmuls accumulating in PSUM; bias added with a
    rank-1 (sqrt(deg) x b) matmul; epilogue relu+LayerNorm on ACT/DVE.
  - Mean pool: one-hot(batch) matmuls with an appended ones column, 33KB
    AllReduce, divide by max(count, 1).
"""

import math
import os
import sys

import numpy as np

for p in ("/opt/trn_rl_repo",):
    if p not in sys.path and os.path.isdir(p):
        sys.path.insert(0, p)

import ml_dtypes

BF16 = ml_dtypes.bfloat16


# ---------------------------------------------------------------------------
# configuration
# ---------------------------------------------------------------------------

class Cfg:
    def __init__(self, N=65536, E=1048576, NG=64, D_IN=256, D_HID=256, D_OUT=128,
                 NQ=2, GROUP=4, EPS=1e-5):
        self.N, self.E, self.NG = N, E, NG
        self.D_IN, self.D_HID, self.D_OUT = D_IN, D_HID, D_OUT
        self.EPS = EPS
        self.NCORES = 8
        self.P = 128
        self.NPC = N // self.NCORES                  # nodes per core
        self.NT = self.NPC // self.P                 # dst tiles per core
        self.NQ = NQ                                 # AllGather chunks (src quarters)
        self.QROWS = self.NPC // NQ                  # local rows per quarter
        self.TABROWS = self.QROWS * self.NCORES      # gather-table rows (int16 bound)
        assert self.TABROWS <= 32768
        self.GROUP = GROUP                           # dst tiles per gather group
        self.NGROUPS = self.NT // GROUP
        self.GPQ = self.NGROUPS // NQ                # groups per quarter
        assert self.NT % GROUP == 0 and self.NGROUPS % NQ == 0
        assert self.QROWS % self.P == 0
        self.DIMS = [(D_IN, D_HID), (D_HID, D_HID), (D_HID, D_OUT)]
        self.CWIN = 8                                # chunks per one-hot build
        self.DBG_NO_GATHER = False                   # crash bisection knobs
        self.DBG_LOCAL_TABLE = False
        self.MAX_CALL_IDX = 1 << 30                  # split big dma_gather calls
        self.SINGLE_PACKET = False  # single_packet=True serializes + faults >2k idx
        self.GBUFS = 2
        self.CWBUFS = 4
        self.CW_GPSIMD = 0  # every Nth one-hot window built on gpsimd (0=never)


# ---------------------------------------------------------------------------
# host-side edge preprocessing (pure integer index work)
# ---------------------------------------------------------------------------

class Prep:
    pass


def prep_edges(cfg, edge_index, batch):
    """Sort/partition edges, build the shared padded segment structure and the
    per-core int16 gather-index / local-dst arrays."""
    c = cfg
    P, NPC, NT, NQ, QROWS, GROUP = c.P, c.NPC, c.NT, c.NQ, c.QROWS, c.GROUP
    src = np.concatenate([np.asarray(edge_index[0], np.int64),
                          np.arange(c.N, dtype=np.int64)])
    dst = np.concatenate([np.asarray(edge_index[1], np.int64),
                          np.arange(c.N, dtype=np.int64)])
    deg = np.bincount(dst, minlength=c.N).astype(np.float32)

    core = dst // NPC
    q_of = (src % NPC) // QROWS                       # AllGather chunk of src
    gidx = (src // NPC) * QROWS + (src % NPC) - q_of * QROWS  # idx in table q

    per_core = []
    counts = np.zeros((c.NCORES, NT, NQ), np.int64)
    for ci in range(c.NCORES):
        m = core == ci
        s_q, s_g, d_l = q_of[m], gidx[m], dst[m] - ci * NPC
        t = d_l // P
        grp = t // GROUP
        order = np.lexsort((t, s_q, grp))             # group -> quarter -> tile
        s_q, s_g, d_l, t = s_q[order], s_g[order], d_l[order], t[order]
        cnt = np.bincount(t * NQ + s_q, minlength=NT * NQ).reshape(NT, NQ)
        counts[ci] = cnt
        per_core.append((s_q, s_g, d_l % P, t))

    pad = ((counts.max(axis=0) + P - 1) // P) * P     # shared [NT, NQ] padded sizes

    # stream layout: for g: for q: for t in g: seg(t, q)
    seg_base = np.zeros((NT, NQ), np.int64)           # slot offset of each segment
    call_base = np.zeros((c.NGROUPS, NQ), np.int64)   # slot offset of each gather call
    call_n = np.zeros((c.NGROUPS, NQ), np.int64)      # num_idxs per call
    group_base = np.zeros(c.NGROUPS, np.int64)
    pos = 0                                           # in slots (chunks of 128)
    for g in range(c.NGROUPS):
        group_base[g] = pos
        for q in range(NQ):
            call_base[g, q] = pos
            for t in range(g * GROUP, (g + 1) * GROUP):
                seg_base[t, q] = pos
                pos += pad[t, q] // P
            call_n[g, q] = (pos - call_base[g, q]) * P
    nchunks = pos
    totidx = nchunks * P

    pr = Prep()
    pr.deg = deg
    pr.pad, pr.seg_base = pad, seg_base
    pr.call_base, pr.call_n, pr.group_base = call_base, call_n, group_base
    pr.nchunks, pr.totidx = nchunks, totidx
    pr.smax = int((np.diff(np.append(group_base, nchunks))).max())
    pr.smax_q = [int(call_n[:, q].max()) // P for q in range(NQ)]
    # ordered chunk list per tile: (quarter, call-local slot) pairs
    pr.tile_slots = [
        [(q, int(s - call_base[t // GROUP, q]))
         for q in range(NQ)
         for s in range(seg_base[t, q], seg_base[t, q] + pad[t, q] // P)]
        for t in range(NT)
    ]

    pr.idx16, pr.dstloc = [], []
    for ci in range(c.NCORES):
        s_q, s_g, dl, t = per_core[ci]
        idx = np.zeros(totidx, np.int16)
        dloc = np.full(totidx, -1.0, np.float32)
        # per-(t,q) segment fill
        start = np.zeros((NT, NQ), np.int64)
        start[:, :] = seg_base * P
        # edges are already in (g, q, t) order; compute per-seg offsets
        key = t * NQ + s_q
        # stable positions within each segment
        seg_begin = (seg_base * P)[t, s_q]
        # rank within segment: since sorted by (grp, q, t), same-(t,q) edges
        # are contiguous; use cumulative count trick
        _, first_pos, inv = np.unique(key, return_index=True, return_inverse=True)
        offs = np.arange(len(key)) - first_pos[inv]
        ppos = seg_begin + offs
        idx[ppos] = s_g.astype(np.int16)
        dloc[ppos] = dl.astype(np.float32)
        cols = totidx // 16
        pr.idx16.append(np.tile(idx.reshape(cols, 16).T, (8, 1)))   # [128, cols]
        pr.dstloc.append(np.ascontiguousarray(
            dloc.reshape(nchunks, P).T.astype(BF16)))               # [128, nchunks]
    return pr


# ---------------------------------------------------------------------------
# bass program
# ---------------------------------------------------------------------------

def build_program(cfg, pr):
    import concourse.bass as bass
    import concourse.mybir as mybir
    import concourse.tile as tile
    from concourse import bacc
    from concourse.masks import make_identity
    from concourse._compat import axon_active

    c = cfg
    P, NT, NQ, NGROUPS, GROUP = c.P, c.NT, c.NQ, c.NGROUPS, c.GROUP
    f32, bf16, i16 = mybir.dt.float32, mybir.dt.bfloat16, mybir.dt.int16
    AF = mybir.ActivationFunctionType
    OP = mybir.AluOpType
    RG = [list(range(c.NCORES))]

    nc = bacc.Bacc("TRN2", target_bir_lowering=False, debug=False,
                   num_devices=c.NCORES)

    # ---- I/O ----
    x_in = nc.dram_tensor("x_c", [c.NPC, c.D_IN], f32, kind="ExternalInput")
    Wd, bd = [], []
    for li, (din, dout) in enumerate(c.DIMS):
        Wd.append(nc.dram_tensor(f"W{li + 1}c", [P, din // P, dout], bf16,
                                 kind="ExternalInput"))
        bd.append(nc.dram_tensor(f"b{li + 1}c", [1, dout], bf16,
                                 kind="ExternalInput"))
    gbe = [nc.dram_tensor(n, [c.D_HID], f32, kind="ExternalInput")
           for n in ("g1c", "be1c", "g2c", "be2c")]
    degc_d = nc.dram_tensor("deg_col", [P, NT], f32, kind="ExternalInput")
    degr_d = nc.dram_tensor("deg_row", [1, c.NPC], f32, kind="ExternalInput")
    batch_d = nc.dram_tensor("batch_col", [P, NT], bf16, kind="ExternalInput")
    idx_d = nc.dram_tensor("idx16", [P, pr.totidx // 16], i16, kind="ExternalInput")
    dstl_d = nc.dram_tensor("dstloc", [P, pr.nchunks], bf16, kind="ExternalInput")
    h_out = nc.dram_tensor("h_out", [c.NPC, c.D_OUT], f32, kind="ExternalOutput")
    pooled_out = nc.dram_tensor("pooled_out", [c.NG, c.D_OUT], f32,
                                kind="ExternalOutput")

    with tile.TileContext(nc, num_cores=c.NCORES) as tc:
      with tc.tile_pool(name="const", bufs=1) as cp, \
           tc.tile_pool(name="hrow", bufs=3) as hrp, \
           tc.tile_pool(name="ht", bufs=3) as htp, \
           tc.tile_pool(name="zs", bufs=4) as zsp, \
           tc.tile_pool(name="gbuf", bufs=c.GBUFS) as gbp, \
           tc.tile_pool(name="cwin", bufs=c.CWBUFS) as cwp, \
           tc.tile_pool(name="l3", bufs=3) as l3p, \
           tc.tile_pool(name="spmm_ps", bufs=4, space="PSUM") as spp, \
           tc.tile_pool(name="z_ps", bufs=2, space="PSUM") as zpp, \
           tc.tile_pool(name="t_ps", bufs=2, space="PSUM") as tpp, \
           tc.tile_pool(name="dram", bufs=1, space="DRAM") as dr:

        # ---- constants ----
        Wt = []
        for li, (din, dout) in enumerate(c.DIMS):
            w = cp.tile([P, din // P, dout], bf16, name=f"Wt{li}")
            nc.sync.dma_start(out=w[:, :, :], in_=Wd[li][:, :, :])
            Wt.append(w)
        brow = []
        for li, (din, dout) in enumerate(c.DIMS):
            b = cp.tile([1, dout], bf16, name=f"brow{li}")
            nc.sync.dma_start(out=b[:, :], in_=bd[li][:, :])
            brow.append(b)
        gb_bc = []
        for gi, gt in enumerate(gbe):
            t_ = cp.tile([P, c.D_HID], f32, name=f"gbbc{gi}")
            bcast = bass.AP(tensor=gt, offset=0, ap=[[0, P], [1, c.D_HID]])
            nc.gpsimd.dma_start(out=t_[:, :], in_=bcast)
            gb_bc.append(t_)
        eps_t = cp.tile([P, 1], f32, name="eps_t")
        nc.vector.memset(eps_t[:, :], c.EPS)
        ident = cp.tile([P, P], f32, name="ident")
        make_identity(nc, ident[:, :])
        iota_i = cp.tile([P, c.CWIN * P], i16, name="iota_i")
        nc.gpsimd.iota(iota_i[:, :], pattern=[[0, c.CWIN], [1, P]], base=0,
                       channel_multiplier=0)
        iota_b = cp.tile([P, c.CWIN, P], bf16, name="iota_b")
        nc.vector.tensor_copy(out=iota_b[:, :, :],
                              in_=iota_i[:, :].rearrange("p (w d) -> p w d",
                                                         w=c.CWIN))
        deg_c = cp.tile([P, NT], f32, name="deg_c")
        nc.sync.dma_start(out=deg_c[:, :], in_=degc_d[:, :])
        dis_c = cp.tile([P, NT], f32, name="dis_c")
        nc.scalar.activation(out=dis_c[:, :], in_=deg_c[:, :], func=AF.Sqrt)
        nc.vector.reciprocal(out=dis_c[:, :], in_=dis_c[:, :])
        deg_r = cp.tile([1, c.NPC], f32, name="deg_r")
        nc.sync.dma_start(out=deg_r[:, :], in_=degr_d[:, :])
        invdis_r = cp.tile([1, c.NPC], bf16, name="invdis_r")
        nc.scalar.activation(out=invdis_r[:, :], in_=deg_r[:, :], func=AF.Sqrt)
        batch_c = cp.tile([P, NT], bf16, name="batch_c")
        nc.sync.dma_start(out=batch_c[:, :], in_=batch_d[:, :])
        idx_t = cp.tile([P, pr.totidx // 16], i16, name="idx_t")
        nc.sync.dma_start(out=idx_t[:, :], in_=idx_d[:, :])
        dstl_t = cp.tile([P, pr.nchunks], bf16, name="dstl_t")
        nc.sync.dma_start(out=dstl_t[:, :], in_=dstl_d[:, :])

        # ---- comm buffers ----
        zsin = [[dr.tile([c.QROWS, c.DIMS[li][1]], bf16, name=f"zsin{li}_{q}",
                         tag=f"zsin{li}_{q}")
                 for q in range(NQ)] for li in range(3)]
        # NOTE: dma_gather from the Shared scratchpad region faults the device
        # (NRT_EXEC_UNIT_UNRECOVERABLE) beyond small sizes; AllGather into a
        # Local internal tile works (bass warns about perf only).
        tab_space = "Shared" if False else "Local"
        zstab = [[dr.tile([c.TABROWS, c.DIMS[li][1]], bf16,
                          name=f"zstab{li}_{q}", tag=f"zstab{li}_{q}",
                          addr_space=tab_space)
                  for q in range(NQ)] for li in range(3)]
        pool_in = dr.tile([c.NG, 132], f32, name="pool_in", tag="pool_in")
        pool_ag = dr.tile([c.NG, 132], f32, name="pool_ag", tag="pool_ag",
                          addr_space="Shared")

        spmm_ps = [None] * NT      # live PSUM tile per dst tile

        def phase_a_tile(li, t_):
            """Produce zs(layer li, one node tile) from h(li-1); li=0 reads x."""
            din, dout = c.DIMS[li]
            if li == 0:
                hrow = hrp.tile([P, c.D_HID], f32, name="hrow", tag="hrow")
                nc.sync.dma_start(out=hrow[:, :din],
                                  in_=x_in[t_ * P:(t_ + 1) * P, :])
            else:
                ps = spmm_ps[t_]
                spmm_ps[t_] = None
                hrow = hrp.tile([P, c.D_HID], f32, name="hrow", tag="hrow")
                nc.scalar.activation(out=hrow[:, :din], in_=ps[:, :din],
                                     func=AF.Relu, scale=dis_c[:, t_:t_ + 1])
                st = hrp.tile([P, 6], f32, name="bnst", tag="bnst")
                nc.vector.bn_stats(out=st[:, :], in_=hrow[:, :din])
                mv = hrp.tile([P, 2], f32, name="bnmv", tag="bnmv")
                nc.vector.bn_aggr(out=mv[:, :], in_=st[:, :])
                rs = hrp.tile([P, 1], f32, name="rstd", tag="rstd")
                nc.scalar.activation(out=rs[:, :], in_=mv[:, 1:2], func=AF.Sqrt,
                                     bias=eps_t[:, :])
                nc.vector.reciprocal(out=rs[:, :], in_=rs[:, :])
                nc.vector.tensor_scalar(out=hrow[:, :din], in0=hrow[:, :din],
                                        scalar1=mv[:, 0:1], scalar2=rs[:, :],
                                        op0=OP.subtract, op1=OP.mult)
                gi = 2 * (li - 1)
                nc.vector.tensor_tensor(out=hrow[:, :din], in0=hrow[:, :din],
                                        in1=gb_bc[gi][:, :din], op=OP.mult)
                nc.vector.tensor_tensor(out=hrow[:, :din], in0=hrow[:, :din],
                                        in1=gb_bc[gi + 1][:, :din], op=OP.add)
            tps = tpp.tile([P, din // P, P], f32, name="tps", tag="tps")
            for ih in range(din // P):
                nc.tensor.transpose(out=tps[:, ih, :],
                                    in_=hrow[:, ih * P:(ih + 1) * P],
                                    identity=ident[:, :])
            hT = htp.tile([P, din // P, P], bf16, name="hT", tag="hT")
            nc.vector.tensor_copy(out=hT[:, :, :], in_=tps[:, :, :])
            zp = zpp.tile([P, c.D_HID], f32, name="zp", tag="zp")
            for ih in range(din // P):
                nc.tensor.matmul(out=zp[:, :dout], lhsT=hT[:, ih, :],
                                 rhs=Wt[li][:, ih, :], start=(ih == 0),
                                 stop=(ih == din // P - 1),
                                 skip_group_check=True)
            zst = zsp.tile([P, c.D_HID], bf16, name="zst", tag="zst")
            nc.scalar.activation(out=zst[:, :dout], in_=zp[:, :dout],
                                 func=AF.Copy, scale=dis_c[:, t_:t_ + 1])
            q = t_ // (NT // NQ)
            r0 = (t_ % (NT // NQ)) * P
            nc.sync.dma_start(out=zsin[li][q][r0:r0 + P, :], in_=zst[:, :dout])

        def ag_maybe(li, g):
            if (g + 1) % c.GPQ == 0:
                q = g // c.GPQ
                nc.gpsimd.collective_compute(
                    "AllGather", OP.bypass, replica_groups=RG,
                    ins=[zsin[li][q][:, :].opt()], outs=[zstab[li][q][:, :].opt()])

        def phase_b_group(li, g):
            """Gather + segment-sum matmuls for dst tiles of group g, layer li."""
            dout = c.DIMS[li][1]
            g0 = int(pr.group_base[g])
            ns = int(pr.group_base[g + 1] if g + 1 < NGROUPS else pr.nchunks) - g0
            # one gather buffer per src-quarter so quarter q's buffers recycle
            # without waiting on quarter q+1's AllGather
            gbq = [gbp.tile([P, pr.smax_q[q], dout], bf16, name=f"gb{q}",
                            tag=f"gb{q}") for q in range(NQ)]
            for q in range(NQ):
                n_all = int(pr.call_n[g, q])
                if c.DBG_NO_GATHER:
                    if n_all:
                        nc.vector.memset(gbq[q][:, :n_all // P, :], 0.001)
                    continue
                src_tab = zstab[li][q]
                for o in range(0, n_all, c.MAX_CALL_IDX):
                    n_idx = min(c.MAX_CALL_IDX, n_all - o)
                    b0 = o // P
                    col0 = (int(pr.call_base[g, q]) * P + o) // 16
                    nc.gpsimd.dma_gather(
                        gbq[q][:, b0:b0 + n_idx // P, :],
                        src_tab[:, :],
                        idx_t[:, col0:col0 + n_idx // 16],
                        n_idx, n_idx, dout,
                        single_packet=c.SINGLE_PACKET)
            cw_of = {}
            for w0 in range(0, ns, c.CWIN):
                wl = min(c.CWIN, ns - w0)
                cw = cwp.tile([P, c.CWIN, P], bf16, name="cw", tag="cw")
                eng = (nc.gpsimd if c.CW_GPSIMD and (w0 // c.CWIN) % c.CW_GPSIMD == 0
                       else nc.vector)
                eng.tensor_tensor(
                    out=cw[:, :wl, :],
                    in0=dstl_t[:, g0 + w0:g0 + w0 + wl, None].broadcast_to(
                        [P, wl, P]),
                    in1=iota_b[:, :wl, :], op=OP.is_equal)
                for j in range(wl):
                    cw_of[g0 + w0 + j] = (cw, j)
            for t_ in range(g * GROUP, (g + 1) * GROUP):
                ps = spp.tile([P, c.D_HID], f32, name="sps", tag="sps")
                spmm_ps[t_] = ps
                slots = pr.tile_slots[t_]
                for k, (q, sl) in enumerate(slots):
                    s = int(pr.call_base[g, q]) + sl
                    cw, j = cw_of[s]
                    nc.tensor.matmul(out=ps[:, :dout], lhsT=cw[:, j, :],
                                     rhs=gbq[q][:, sl, :], start=(k == 0),
                                     stop=False, skip_group_check=True)
                nc.tensor.matmul(out=ps[:, :dout],
                                 lhsT=invdis_r[:, t_ * P:(t_ + 1) * P],
                                 rhs=brow[li][:, :], start=False, stop=True,
                                 skip_group_check=True)

        # ---- bootstrap: phase A of layer 1 from x ----
        for g in range(NGROUPS):
            for t_ in range(g * GROUP, (g + 1) * GROUP):
                phase_a_tile(0, t_)
            ag_maybe(0, g)

        pool_ps = None
        for li in range(3):
            dout = c.DIMS[li][1]
            if li == 2:
                pool_ps = zpp.tile([c.NG, 132], f32, name="poolps", tag="zp")
            for g in range(NGROUPS):
                phase_b_group(li, g)
                if li < 2:
                    for t_ in range(g * GROUP, (g + 1) * GROUP):
                        phase_a_tile(li + 1, t_)
                    ag_maybe(li + 1, g)
                else:
                    for t_ in range(g * GROUP, (g + 1) * GROUP):
                        ps = spmm_ps[t_]
                        spmm_ps[t_] = None
                        h3 = hrp.tile([P, c.D_HID], f32, name="hrow", tag="hrow")
                        nc.scalar.activation(out=h3[:, :dout], in_=ps[:, :dout],
                                             func=AF.Copy,
                                             scale=dis_c[:, t_:t_ + 1])
                        nc.sync.dma_start(out=h_out[t_ * P:(t_ + 1) * P, :],
                                          in_=h3[:, :dout])
                        h3b = l3p.tile([P, dout + 4], bf16, name="h3b", tag="h3b")
                        nc.vector.tensor_copy(out=h3b[:, :dout], in_=h3[:, :dout])
                        nc.vector.memset(h3b[:, dout:dout + 1], 1.0)
                        oh = l3p.tile([P, c.NG], bf16, name="oh", tag="oh")
                        nc.vector.tensor_tensor(
                            out=oh[:, :],
                            in0=batch_c[:, t_:t_ + 1].broadcast_to([P, c.NG]),
                            in1=iota_b[:, 0, :c.NG], op=OP.is_equal)
                        nc.tensor.matmul(out=pool_ps[:, :dout + 1],
                                         lhsT=oh[:, :], rhs=h3b[:, :dout + 1],
                                         start=(t_ == 0), stop=(t_ == NT - 1),
                                         skip_group_check=True)

        # ---- pooled tail ----
        psb = cp.tile([c.NG, 132], f32, name="psb")
        nc.vector.memset(psb[:, :], 0.0)
        nc.vector.tensor_copy(out=psb[:, :c.D_OUT + 1],
                              in_=pool_ps[:, :c.D_OUT + 1])
        nc.sync.dma_start(out=pool_in[:, :], in_=psb[:, :])
        nc.gpsimd.collective_compute(
            "AllReduce", mybir_add(nc), replica_groups=RG,
            ins=[pool_in[:, :].opt()], outs=[pool_ag[:, :].opt()])
        pres = cp.tile([c.NG, 132], f32, name="pres")
        nc.sync.dma_start(out=pres[:, :], in_=pool_ag[:, :])
        cnt = cp.tile([c.NG, 1], f32, name="cnt")
        nc.vector.tensor_scalar(out=cnt[:, :], in0=pres[:, c.D_OUT:c.D_OUT + 1],
                                scalar1=1.0, scalar2=None, op0=OP.max)
        nc.vector.reciprocal(out=cnt[:, :], in_=cnt[:, :])
        pooled = cp.tile([c.NG, c.D_OUT], f32, name="pooled")
        nc.vector.tensor_scalar(out=pooled[:, :], in0=pres[:, :c.D_OUT],
                                scalar1=cnt[:, :], scalar2=None, op0=OP.mult)
        nc.sync.dma_start(out=pooled_out[:, :], in_=pooled[:, :])

    nc.compile()
    return nc


def mybir_add(nc):
    import concourse.mybir as mybir
    return mybir.AluOpType.add


# ---------------------------------------------------------------------------
# per-core input maps
# ---------------------------------------------------------------------------

def make_in_maps(cfg, pr, inputs):
    c = cfg
    x = np.asarray(inputs["x"], np.float32)
    batch = np.asarray(inputs["batch"], np.int64)
    Ws = [np.asarray(inputs[k], np.float32) for k in ("W1", "W2", "W3")]
    bs = [np.asarray(inputs[k], np.float32) for k in ("b1", "b2", "b3")]
    gs = [np.asarray(inputs[k], np.float32) for k in ("g1", "be1", "g2", "be2")]

    maps = []
    for ci in range(c.NCORES):
        sl = slice(ci * c.NPC, (ci + 1) * c.NPC)
        m = {"x_c": np.ascontiguousarray(x[sl])}
        for li in range(3):
            W = Ws[li]
            din, dout = c.DIMS[li]
            m[f"W{li + 1}c"] = np.ascontiguousarray(
                W.reshape(din // c.P, c.P, dout).transpose(1, 0, 2).astype(BF16))
            m[f"b{li + 1}c"] = bs[li].reshape(1, -1).astype(BF16)
        for gi, n in enumerate(("g1c", "be1c", "g2c", "be2c")):
            m[n] = np.ascontiguousarray(gs[gi])
        dg = pr.deg[sl]
        m["deg_col"] = np.ascontiguousarray(dg.reshape(c.NT, c.P).T)
        m["deg_row"] = np.ascontiguousarray(dg.reshape(1, c.NPC))
        m["batch_col"] = np.ascontiguousarray(
            batch[sl].reshape(c.NT, c.P).T.astype(BF16))
        m["idx16"] = np.ascontiguousarray(pr.idx16[ci])
        m["dstloc"] = np.ascontiguousarray(pr.dstloc[ci])
        maps.append(m)
    return maps


# ---------------------------------------------------------------------------
# entry points
# ---------------------------------------------------------------------------

_CACHE = {}


def _build(inputs, cfg=None):
    cfg = cfg or Cfg()
    key = (cfg.N, cfg.E, cfg.NG, cfg.NQ, cfg.GROUP)
    if key not in _CACHE:
        pr = prep_edges(cfg, np.asarray(inputs["edge_index"], np.int64),
                        np.asarray(inputs["batch"], np.int64))
        nc = build_program(cfg, pr)
        _CACHE[key] = (cfg, pr, nc)
    cfg, pr, nc = _CACHE[key]
    in_maps = make_in_maps(cfg, pr, inputs)
    return cfg, pr, nc, in_maps


def _run(inputs, trace=False, cfg=None):
    from concourse import bass_utils
    cfg, pr, nc, in_maps = _build(inputs, cfg)
    res = bass_utils.run_bass_kernel_spmd(
        nc, in_maps, core_ids=list(range(cfg.NCORES)), trace=False)
    h = np.concatenate([r["h_out"] for r in res.results], axis=0)
    pooled = res.results[0]["pooled_out"]
    return (h.astype(np.float32), pooled.astype(np.float32)), res


def bench(inputs, iters=10, cfg=None):
    """Build the sharded PJRT executable once, keep inputs device-resident,
    and wall-clock repeated executions. Returns ((h, pooled), best_ns)."""
    import time

    import jax
    from jax.sharding import Mesh, NamedSharding, PartitionSpec
    try:
        from jax.experimental.shard_map import shard_map
    except ImportError:
        from jax.sharding import shard_map
    import concourse.mybir as mybir
    from concourse import bass2jax

    cfg, pr, nc, in_maps = _build(inputs, cfg)
    n_cores = cfg.NCORES
    bass2jax.install_neuronx_cc_hook()

    partition_name = (nc.partition_id_tensor.name
                      if nc.partition_id_tensor else None)
    in_names, out_names, out_avals, zero_outs = [], [], [], []
    for alloc in nc.m.functions[0].allocations:
        if not isinstance(alloc, mybir.MemoryLocationSet):
            continue
        name = alloc.memorylocations[0].name
        if alloc.kind == "ExternalInput":
            if name != partition_name:
                in_names.append(name)
        elif alloc.kind == "ExternalOutput":
            shape = tuple(alloc.tensor_shape)
            dtype = mybir.dt.np(alloc.dtype)
            out_names.append(name)
            out_avals.append(jax.core.ShapedArray(shape, dtype))
            zero_outs.append(np.zeros(shape, dtype))
    n_params = len(in_names)
    all_in_names = in_names + out_names
    if partition_name is not None:
        all_in_names = all_in_names + [partition_name]

    def _exec_once(ins, zeros):
        operands = list(ins) + list(zeros)
        if partition_name is not None:
            operands.append(bass2jax.partition_id_tensor())
        outs = bass2jax._bass_exec_p.bind(
            *operands,
            out_avals=tuple(out_avals),
            in_names=tuple(all_in_names),
            out_names=tuple(out_names),
            lowering_input_output_aliases=(),
            sim_require_finite=True,
            sim_require_nnan=True,
            nc=nc,
        )
        return tuple(outs)

    chain = int(os.environ.get("BENCH_CHAIN", "1"))

    def _body(*args):
        ins, zeros = args[:n_params], args[n_params:]
        outs = _exec_once(ins, zeros)
        for _ in range(chain - 1):
            outs = _exec_once(ins, outs)
        return outs

    devices = jax.devices()[:n_cores]
    mesh = Mesh(np.asarray(devices), ("core",))
    spec = PartitionSpec("core")
    sharded = jax.jit(
        shard_map(_body, mesh=mesh, in_specs=(spec,) * (n_params + len(out_names)),
                  out_specs=(spec,) * len(out_names), check_rep=False),
        keep_unused=True)
    sh = NamedSharding(mesh, spec)
    concat_in = [
        jax.device_put(
            np.concatenate([np.asarray(in_maps[c][n]) for c in range(n_cores)],
                           axis=0), sh)
        for n in in_names
    ]
    concat_zeros = [
        jax.device_put(np.zeros((n_cores * z.shape[0], *z.shape[1:]), z.dtype), sh)
        for z in zero_outs
    ]
    out = sharded(*concat_in, *concat_zeros)
    jax.block_until_ready(out)
    times = []
    for _ in range(iters):
        t0 = time.perf_counter()
        out = sharded(*concat_in, *concat_zeros)
        jax.block_until_ready(out)
        times.append(time.perf_counter() - t0)
    best_ns = int(min(times) * 1e9)
    outs = {n: np.asarray(out[i]) for i, n in enumerate(out_names)}
    h = outs["h_out"].reshape(n_cores, cfg.NPC, cfg.D_OUT).reshape(-1, cfg.D_OUT)
    pooled = outs["pooled_out"].reshape(n_cores, cfg.NG, cfg.D_OUT)[0]
    print("bench times (ms):", [f"{t * 1e3:.3f}" for t in times])
    return (h.astype(np.float32), pooled.astype(np.float32)), best_ns


def kernel(**inputs):
    (h, pooled), _ = _run(inputs, trace=False)
    return h, pooled


# revision 25
# speedup vs baseline: 1205.4646x; 3.6998x over previous
"""GCN encoder (3-layer message passing + LayerNorm + mean pool) on 8 trn2 cores.

Strategy (see also spec sharding_hint):
  - Nodes partitioned 8 ways by id (dst-sharding). Self loops appended as
    ordinary edges. Per core, edges grouped by (dst-tile of 128, src-quarter)
    with a *shared* padded structure (max over cores) so a single SPMD NEFF
    serves all cores; per-core data = int16 gather indices + local-dst ids.
  - Per layer: dense z = h @ W on PE (activations PE-transposed on the fly),
    z scaled by deg^-1/2, cast bf16, AllGather'd in 4 node-quarter chunks
    (tables of 16384 rows keep dma_gather's int16 indices in range and let
    gathers of quarter q overlap the AllGather of quarter q+1).
  - Message passing: per-edge dma_gather of 512B rows into SBUF, segment-sum
    via one-hot(dst_local) matmuls accumulating in PSUM; bias added with a
    rank-1 (sqrt(deg) x b) matmul; epilogue relu+LayerNorm on ACT/DVE.
  - Mean pool: one-hot(batch) matmuls with an appended ones column, 33KB
    AllReduce, divide by max(count, 1).
"""

import math
import os
import sys

import numpy as np

for p in ("/opt/trn_rl_repo",):
    if p not in sys.path and os.path.isdir(p):
        sys.path.insert(0, p)

import ml_dtypes

BF16 = ml_dtypes.bfloat16


# ---------------------------------------------------------------------------
# configuration
# ---------------------------------------------------------------------------

class Cfg:
    def __init__(self, N=65536, E=1048576, NG=64, D_IN=256, D_HID=256, D_OUT=128,
                 NQ=2, GROUP=2, EPS=1e-5):
        self.N, self.E, self.NG = N, E, NG
        self.D_IN, self.D_HID, self.D_OUT = D_IN, D_HID, D_OUT
        self.EPS = EPS
        self.NCORES = 8
        self.P = 128
        self.NPC = N // self.NCORES                  # nodes per core
        self.NT = self.NPC // self.P                 # dst tiles per core
        self.NQ = NQ                                 # AllGather chunks (src quarters)
        self.QROWS = self.NPC // NQ                  # local rows per quarter
        self.TABROWS = self.QROWS * self.NCORES      # gather-table rows (int16 bound)
        assert self.TABROWS <= 32768
        self.GROUP = GROUP                           # dst tiles per gather group
        self.NGROUPS = self.NT // GROUP
        self.GPQ = self.NGROUPS // NQ                # groups per quarter
        assert self.NT % GROUP == 0 and self.NGROUPS % NQ == 0
        assert self.QROWS % self.P == 0
        self.DIMS = [(D_IN, D_HID), (D_HID, D_HID), (D_HID, D_OUT)]
        self.CWIN = 8                                # chunks per one-hot build
        self.DBG_NO_GATHER = False                   # crash bisection knobs
        self.DBG_LOCAL_TABLE = False
        self.MAX_CALL_IDX = 1 << 30                  # split big dma_gather calls
        self.SINGLE_PACKET = False  # single_packet=True serializes + faults >2k idx
        self.GBUFS = 3
        self.CWBUFS = 4
        self.CW_GPSIMD = 0  # gpsimd TensorTensor rejected by walrus codegen


# ---------------------------------------------------------------------------
# host-side edge preprocessing (pure integer index work)
# ---------------------------------------------------------------------------

class Prep:
    pass


def prep_edges(cfg, edge_index, batch):
    """Sort/partition edges, build the shared padded segment structure and the
    per-core int16 gather-index / local-dst arrays."""
    c = cfg
    P, NPC, NT, NQ, QROWS, GROUP = c.P, c.NPC, c.NT, c.NQ, c.QROWS, c.GROUP
    src = np.concatenate([np.asarray(edge_index[0], np.int64),
                          np.arange(c.N, dtype=np.int64)])
    dst = np.concatenate([np.asarray(edge_index[1], np.int64),
                          np.arange(c.N, dtype=np.int64)])
    deg = np.bincount(dst, minlength=c.N).astype(np.float32)

    core = dst // NPC
    q_of = (src % NPC) // QROWS                       # AllGather chunk of src
    gidx = (src // NPC) * QROWS + (src % NPC) - q_of * QROWS  # idx in table q

    per_core = []
    counts = np.zeros((c.NCORES, NT, NQ), np.int64)
    for ci in range(c.NCORES):
        m = core == ci
        s_q, s_g, d_l = q_of[m], gidx[m], dst[m] - ci * NPC
        t = d_l // P
        grp = t // GROUP
        order = np.lexsort((t, s_q, grp))             # group -> quarter -> tile
        s_q, s_g, d_l, t = s_q[order], s_g[order], d_l[order], t[order]
        cnt = np.bincount(t * NQ + s_q, minlength=NT * NQ).reshape(NT, NQ)
        counts[ci] = cnt
        per_core.append((s_q, s_g, d_l % P, t))

    pad = ((counts.max(axis=0) + P - 1) // P) * P     # shared [NT, NQ] padded sizes

    # stream layout: for g: for q: for t in g: seg(t, q)
    seg_base = np.zeros((NT, NQ), np.int64)           # slot offset of each segment
    call_base = np.zeros((c.NGROUPS, NQ), np.int64)   # slot offset of each gather call
    call_n = np.zeros((c.NGROUPS, NQ), np.int64)      # num_idxs per call
    group_base = np.zeros(c.NGROUPS, np.int64)
    pos = 0                                           # in slots (chunks of 128)
    for g in range(c.NGROUPS):
        group_base[g] = pos
        for q in range(NQ):
            call_base[g, q] = pos
            for t in range(g * GROUP, (g + 1) * GROUP):
                seg_base[t, q] = pos
                pos += pad[t, q] // P
            call_n[g, q] = (pos - call_base[g, q]) * P
    nchunks = pos
    totidx = nchunks * P

    pr = Prep()
    pr.deg = deg
    pr.pad, pr.seg_base = pad, seg_base
    pr.call_base, pr.call_n, pr.group_base = call_base, call_n, group_base
    pr.nchunks, pr.totidx = nchunks, totidx
    pr.smax = int((np.diff(np.append(group_base, nchunks))).max())
    pr.smax_q = [int(call_n[:, q].max()) // P for q in range(NQ)]
    # ordered chunk list per tile: (quarter, call-local slot) pairs
    pr.tile_slots = [
        [(q, int(s - call_base[t // GROUP, q]))
         for q in range(NQ)
         for s in range(seg_base[t, q], seg_base[t, q] + pad[t, q] // P)]
        for t in range(NT)
    ]

    pr.idx16, pr.dstloc = [], []
    for ci in range(c.NCORES):
        s_q, s_g, dl, t = per_core[ci]
        idx = np.zeros(totidx, np.int16)
        dloc = np.full(totidx, -1.0, np.float32)
        # per-(t,q) segment fill
        start = np.zeros((NT, NQ), np.int64)
        start[:, :] = seg_base * P
        # edges are already in (g, q, t) order; compute per-seg offsets
        key = t * NQ + s_q
        # stable positions within each segment
        seg_begin = (seg_base * P)[t, s_q]
        # rank within segment: since sorted by (grp, q, t), same-(t,q) edges
        # are contiguous; use cumulative count trick
        _, first_pos, inv = np.unique(key, return_index=True, return_inverse=True)
        offs = np.arange(len(key)) - first_pos[inv]
        ppos = seg_begin + offs
        idx[ppos] = s_g.astype(np.int16)
        dloc[ppos] = dl.astype(np.float32)
        cols = totidx // 16
        pr.idx16.append(np.tile(idx.reshape(cols, 16).T, (8, 1)))   # [128, cols]
        pr.dstloc.append(np.ascontiguousarray(
            dloc.reshape(nchunks, P).T.astype(BF16)))               # [128, nchunks]
    return pr


# ---------------------------------------------------------------------------
# bass program
# ---------------------------------------------------------------------------

def build_program(cfg, pr):
    import concourse.bass as bass
    import concourse.mybir as mybir
    import concourse.tile as tile
    from concourse import bacc
    from concourse.masks import make_identity
    from concourse._compat import axon_active

    c = cfg
    P, NT, NQ, NGROUPS, GROUP = c.P, c.NT, c.NQ, c.NGROUPS, c.GROUP
    f32, bf16, i16 = mybir.dt.float32, mybir.dt.bfloat16, mybir.dt.int16
    AF = mybir.ActivationFunctionType
    OP = mybir.AluOpType
    RG = [list(range(c.NCORES))]

    nc = bacc.Bacc("TRN2", target_bir_lowering=False, debug=False,
                   num_devices=c.NCORES)

    # ---- I/O ----
    x_in = nc.dram_tensor("x_c", [c.NPC, c.D_IN], f32, kind="ExternalInput")
    Wd, bd = [], []
    for li, (din, dout) in enumerate(c.DIMS):
        Wd.append(nc.dram_tensor(f"W{li + 1}c", [P, din // P, dout], bf16,
                                 kind="ExternalInput"))
        bd.append(nc.dram_tensor(f"b{li + 1}c", [1, dout], bf16,
                                 kind="ExternalInput"))
    gbe = [nc.dram_tensor(n, [c.D_HID], f32, kind="ExternalInput")
           for n in ("g1c", "be1c", "g2c", "be2c")]
    degc_d = nc.dram_tensor("deg_col", [P, NT], f32, kind="ExternalInput")
    degr_d = nc.dram_tensor("deg_row", [1, c.NPC], f32, kind="ExternalInput")
    batch_d = nc.dram_tensor("batch_col", [P, NT], bf16, kind="ExternalInput")
    idx_d = nc.dram_tensor("idx16", [P, pr.totidx // 16], i16, kind="ExternalInput")
    dstl_d = nc.dram_tensor("dstloc", [P, pr.nchunks], bf16, kind="ExternalInput")
    h_out = nc.dram_tensor("h_out", [c.NPC, c.D_OUT], f32, kind="ExternalOutput")
    pooled_out = nc.dram_tensor("pooled_out", [c.NG, c.D_OUT], f32,
                                kind="ExternalOutput")

    with tile.TileContext(nc, num_cores=c.NCORES) as tc:
      with tc.tile_pool(name="const", bufs=1) as cp, \
           tc.tile_pool(name="hrow", bufs=3) as hrp, \
           tc.tile_pool(name="ht", bufs=3) as htp, \
           tc.tile_pool(name="zs", bufs=4) as zsp, \
           tc.tile_pool(name="gbuf", bufs=c.GBUFS) as gbp, \
           tc.tile_pool(name="cwin", bufs=c.CWBUFS) as cwp, \
           tc.tile_pool(name="l3", bufs=3) as l3p, \
           tc.tile_pool(name="spmm_ps", bufs=4, space="PSUM") as spp, \
           tc.tile_pool(name="z_ps", bufs=2, space="PSUM") as zpp, \
           tc.tile_pool(name="t_ps", bufs=2, space="PSUM") as tpp, \
           tc.tile_pool(name="dram", bufs=1, space="DRAM") as dr:

        # ---- constants ----
        Wt = []
        for li, (din, dout) in enumerate(c.DIMS):
            w = cp.tile([P, din // P, dout], bf16, name=f"Wt{li}")
            nc.sync.dma_start(out=w[:, :, :], in_=Wd[li][:, :, :])
            Wt.append(w)
        brow = []
        for li, (din, dout) in enumerate(c.DIMS):
            b = cp.tile([1, dout], bf16, name=f"brow{li}")
            nc.sync.dma_start(out=b[:, :], in_=bd[li][:, :])
            brow.append(b)
        gb_bc = []
        for gi, gt in enumerate(gbe):
            t_ = cp.tile([P, c.D_HID], f32, name=f"gbbc{gi}")
            bcast = bass.AP(tensor=gt, offset=0, ap=[[0, P], [1, c.D_HID]])
            nc.gpsimd.dma_start(out=t_[:, :], in_=bcast)
            gb_bc.append(t_)
        eps_t = cp.tile([P, 1], f32, name="eps_t")
        nc.vector.memset(eps_t[:, :], c.EPS)
        ident = cp.tile([P, P], f32, name="ident")
        make_identity(nc, ident[:, :])
        iota_i = cp.tile([P, c.CWIN * P], i16, name="iota_i")
        nc.gpsimd.iota(iota_i[:, :], pattern=[[0, c.CWIN], [1, P]], base=0,
                       channel_multiplier=0)
        iota_b = cp.tile([P, c.CWIN, P], bf16, name="iota_b")
        nc.vector.tensor_copy(out=iota_b[:, :, :],
                              in_=iota_i[:, :].rearrange("p (w d) -> p w d",
                                                         w=c.CWIN))
        deg_c = cp.tile([P, NT], f32, name="deg_c")
        nc.sync.dma_start(out=deg_c[:, :], in_=degc_d[:, :])
        dis_c = cp.tile([P, NT], f32, name="dis_c")
        nc.scalar.activation(out=dis_c[:, :], in_=deg_c[:, :], func=AF.Sqrt)
        nc.vector.reciprocal(out=dis_c[:, :], in_=dis_c[:, :])
        deg_r = cp.tile([1, c.NPC], f32, name="deg_r")
        nc.sync.dma_start(out=deg_r[:, :], in_=degr_d[:, :])
        invdis_r = cp.tile([1, c.NPC], bf16, name="invdis_r")
        nc.scalar.activation(out=invdis_r[:, :], in_=deg_r[:, :], func=AF.Sqrt)
        batch_c = cp.tile([P, NT], bf16, name="batch_c")
        nc.sync.dma_start(out=batch_c[:, :], in_=batch_d[:, :])
        idx_t = cp.tile([P, pr.totidx // 16], i16, name="idx_t")
        nc.sync.dma_start(out=idx_t[:, :], in_=idx_d[:, :])
        dstl_t = cp.tile([P, pr.nchunks], bf16, name="dstl_t")
        nc.sync.dma_start(out=dstl_t[:, :], in_=dstl_d[:, :])

        # ---- comm buffers ----
        zsin = [[dr.tile([c.QROWS, c.DIMS[li][1]], bf16, name=f"zsin{li}_{q}",
                         tag=f"zsin{li}_{q}")
                 for q in range(NQ)] for li in range(3)]
        # NOTE: dma_gather from the Shared scratchpad region faults the device
        # (NRT_EXEC_UNIT_UNRECOVERABLE) beyond small sizes; AllGather into a
        # Local internal tile works (bass warns about perf only).
        tab_space = "Shared" if False else "Local"
        zstab = [[dr.tile([c.TABROWS, c.DIMS[li][1]], bf16,
                          name=f"zstab{li}_{q}", tag=f"zstab{li}_{q}",
                          addr_space=tab_space)
                  for q in range(NQ)] for li in range(3)]
        pool_in = dr.tile([c.NG, 132], f32, name="pool_in", tag="pool_in")
        pool_ag = dr.tile([c.NG, 132], f32, name="pool_ag", tag="pool_ag",
                          addr_space="Shared")

        spmm_ps = [None] * NT      # live PSUM tile per dst tile

        def phase_a_tile(li, t_):
            """Produce zs(layer li, one node tile) from h(li-1); li=0 reads x."""
            din, dout = c.DIMS[li]
            if li == 0:
                hrow = hrp.tile([P, c.D_HID], f32, name="hrow", tag="hrow")
                nc.sync.dma_start(out=hrow[:, :din],
                                  in_=x_in[t_ * P:(t_ + 1) * P, :])
            else:
                ps = spmm_ps[t_]
                spmm_ps[t_] = None
                hrow = hrp.tile([P, c.D_HID], f32, name="hrow", tag="hrow")
                nc.scalar.activation(out=hrow[:, :din], in_=ps[:, :din],
                                     func=AF.Relu, scale=dis_c[:, t_:t_ + 1])
                st = hrp.tile([P, 6], f32, name="bnst", tag="bnst")
                nc.vector.bn_stats(out=st[:, :], in_=hrow[:, :din])
                mv = hrp.tile([P, 2], f32, name="bnmv", tag="bnmv")
                nc.vector.bn_aggr(out=mv[:, :], in_=st[:, :])
                rs = hrp.tile([P, 1], f32, name="rstd", tag="rstd")
                nc.scalar.activation(out=rs[:, :], in_=mv[:, 1:2], func=AF.Sqrt,
                                     bias=eps_t[:, :])
                nc.vector.reciprocal(out=rs[:, :], in_=rs[:, :])
                nc.vector.tensor_scalar(out=hrow[:, :din], in0=hrow[:, :din],
                                        scalar1=mv[:, 0:1], scalar2=rs[:, :],
                                        op0=OP.subtract, op1=OP.mult)
                gi = 2 * (li - 1)
                nc.vector.tensor_tensor(out=hrow[:, :din], in0=hrow[:, :din],
                                        in1=gb_bc[gi][:, :din], op=OP.mult)
                nc.vector.tensor_tensor(out=hrow[:, :din], in0=hrow[:, :din],
                                        in1=gb_bc[gi + 1][:, :din], op=OP.add)
            tps = tpp.tile([P, din // P, P], f32, name="tps", tag="tps")
            for ih in range(din // P):
                nc.tensor.transpose(out=tps[:, ih, :],
                                    in_=hrow[:, ih * P:(ih + 1) * P],
                                    identity=ident[:, :])
            hT = htp.tile([P, din // P, P], bf16, name="hT", tag="hT")
            # ACT copy keeps this off the DVE, which is saturated during the
            # message-passing bursts
            nc.scalar.activation(out=hT[:, :, :], in_=tps[:, :, :], func=AF.Copy)
            zp = zpp.tile([P, c.D_HID], f32, name="zp", tag="zp")
            for ih in range(din // P):
                nc.tensor.matmul(out=zp[:, :dout], lhsT=hT[:, ih, :],
                                 rhs=Wt[li][:, ih, :], start=(ih == 0),
                                 stop=(ih == din // P - 1),
                                 skip_group_check=True)
            zst = zsp.tile([P, c.D_HID], bf16, name="zst", tag="zst")
            nc.scalar.activation(out=zst[:, :dout], in_=zp[:, :dout],
                                 func=AF.Copy, scale=dis_c[:, t_:t_ + 1])
            q = t_ // (NT // NQ)
            r0 = (t_ % (NT // NQ)) * P
            nc.sync.dma_start(out=zsin[li][q][r0:r0 + P, :], in_=zst[:, :dout])

        def ag_maybe(li, g):
            if (g + 1) % c.GPQ == 0:
                q = g // c.GPQ
                nc.gpsimd.collective_compute(
                    "AllGather", OP.bypass, replica_groups=RG,
                    ins=[zsin[li][q][:, :].opt()], outs=[zstab[li][q][:, :].opt()])

        def phase_b_group(li, g):
            """Gather + segment-sum matmuls for dst tiles of group g, layer li."""
            dout = c.DIMS[li][1]
            g0 = int(pr.group_base[g])
            ns = int(pr.group_base[g + 1] if g + 1 < NGROUPS else pr.nchunks) - g0
            # one gather buffer per src-quarter so quarter q's buffers recycle
            # without waiting on quarter q+1's AllGather
            gbq = [gbp.tile([P, pr.smax_q[q], dout], bf16, name=f"gb{q}",
                            tag=f"gb{q}") for q in range(NQ)]
            for q in range(NQ):
                n_all = int(pr.call_n[g, q])
                if c.DBG_NO_GATHER:
                    if n_all:
                        nc.vector.memset(gbq[q][:, :n_all // P, :], 0.001)
                    continue
                src_tab = zstab[li][q]
                for o in range(0, n_all, c.MAX_CALL_IDX):
                    n_idx = min(c.MAX_CALL_IDX, n_all - o)
                    b0 = o // P
                    col0 = (int(pr.call_base[g, q]) * P + o) // 16
                    nc.gpsimd.dma_gather(
                        gbq[q][:, b0:b0 + n_idx // P, :],
                        src_tab[:, :],
                        idx_t[:, col0:col0 + n_idx // 16],
                        n_idx, n_idx, dout,
                        single_packet=c.SINGLE_PACKET)
            cw_of = {}
            for w0 in range(0, ns, c.CWIN):
                wl = min(c.CWIN, ns - w0)
                cw = cwp.tile([P, c.CWIN, P], bf16, name="cw", tag="cw")
                eng = (nc.gpsimd if c.CW_GPSIMD and (w0 // c.CWIN) % c.CW_GPSIMD == 0
                       else nc.vector)
                eng.tensor_tensor(
                    out=cw[:, :wl, :],
                    in0=dstl_t[:, g0 + w0:g0 + w0 + wl, None].broadcast_to(
                        [P, wl, P]),
                    in1=iota_b[:, :wl, :], op=OP.is_equal)
                for j in range(wl):
                    cw_of[g0 + w0 + j] = (cw, j)
            for t_ in range(g * GROUP, (g + 1) * GROUP):
                ps = spp.tile([P, c.D_HID], f32, name="sps", tag="sps")
                spmm_ps[t_] = ps
                slots = pr.tile_slots[t_]
                for k, (q, sl) in enumerate(slots):
                    s = int(pr.call_base[g, q]) + sl
                    cw, j = cw_of[s]
                    nc.tensor.matmul(out=ps[:, :dout], lhsT=cw[:, j, :],
                                     rhs=gbq[q][:, sl, :], start=(k == 0),
                                     stop=False, skip_group_check=True)
                nc.tensor.matmul(out=ps[:, :dout],
                                 lhsT=invdis_r[:, t_ * P:(t_ + 1) * P],
                                 rhs=brow[li][:, :], start=False, stop=True,
                                 skip_group_check=True)

        # ---- bootstrap: phase A of layer 1 from x ----
        for g in range(NGROUPS):
            for t_ in range(g * GROUP, (g + 1) * GROUP):
                phase_a_tile(0, t_)
            ag_maybe(0, g)

        pool_ps = None
        for li in range(3):
            dout = c.DIMS[li][1]
            if li == 2:
                pool_ps = zpp.tile([c.NG, 132], f32, name="poolps", tag="zp")
            for g in range(NGROUPS):
                phase_b_group(li, g)
                if li < 2:
                    for t_ in range(g * GROUP, (g + 1) * GROUP):
                        phase_a_tile(li + 1, t_)
                    ag_maybe(li + 1, g)
                else:
                    for t_ in range(g * GROUP, (g + 1) * GROUP):
                        ps = spmm_ps[t_]
                        spmm_ps[t_] = None
                        h3 = hrp.tile([P, c.D_HID], f32, name="hrow", tag="hrow")
                        nc.scalar.activation(out=h3[:, :dout], in_=ps[:, :dout],
                                             func=AF.Copy,
                                             scale=dis_c[:, t_:t_ + 1])
                        nc.sync.dma_start(out=h_out[t_ * P:(t_ + 1) * P, :],
                                          in_=h3[:, :dout])
                        h3b = l3p.tile([P, dout + 4], bf16, name="h3b", tag="h3b")
                        nc.vector.tensor_copy(out=h3b[:, :dout], in_=h3[:, :dout])
                        nc.vector.memset(h3b[:, dout:dout + 1], 1.0)
                        oh = l3p.tile([P, c.NG], bf16, name="oh", tag="oh")
                        nc.vector.tensor_tensor(
                            out=oh[:, :],
                            in0=batch_c[:, t_:t_ + 1].broadcast_to([P, c.NG]),
                            in1=iota_b[:, 0, :c.NG], op=OP.is_equal)
                        nc.tensor.matmul(out=pool_ps[:, :dout + 1],
                                         lhsT=oh[:, :], rhs=h3b[:, :dout + 1],
                                         start=(t_ == 0), stop=(t_ == NT - 1),
                                         skip_group_check=True)

        # ---- pooled tail ----
        psb = cp.tile([c.NG, 132], f32, name="psb")
        nc.vector.memset(psb[:, :], 0.0)
        nc.vector.tensor_copy(out=psb[:, :c.D_OUT + 1],
                              in_=pool_ps[:, :c.D_OUT + 1])
        nc.sync.dma_start(out=pool_in[:, :], in_=psb[:, :])
        nc.gpsimd.collective_compute(
            "AllReduce", mybir_add(nc), replica_groups=RG,
            ins=[pool_in[:, :].opt()], outs=[pool_ag[:, :].opt()])
        pres = cp.tile([c.NG, 132], f32, name="pres")
        nc.sync.dma_start(out=pres[:, :], in_=pool_ag[:, :])
        cnt = cp.tile([c.NG, 1], f32, name="cnt")
        nc.vector.tensor_scalar(out=cnt[:, :], in0=pres[:, c.D_OUT:c.D_OUT + 1],
                                scalar1=1.0, scalar2=None, op0=OP.max)
        nc.vector.reciprocal(out=cnt[:, :], in_=cnt[:, :])
        pooled = cp.tile([c.NG, c.D_OUT], f32, name="pooled")
        nc.vector.tensor_scalar(out=pooled[:, :], in0=pres[:, :c.D_OUT],
                                scalar1=cnt[:, :], scalar2=None, op0=OP.mult)
        nc.sync.dma_start(out=pooled_out[:, :], in_=pooled[:, :])

    nc.compile()
    return nc


def mybir_add(nc):
    import concourse.mybir as mybir
    return mybir.AluOpType.add


# ---------------------------------------------------------------------------
# per-core input maps
# ---------------------------------------------------------------------------

def make_in_maps(cfg, pr, inputs):
    c = cfg
    x = np.asarray(inputs["x"], np.float32)
    batch = np.asarray(inputs["batch"], np.int64)
    Ws = [np.asarray(inputs[k], np.float32) for k in ("W1", "W2", "W3")]
    bs = [np.asarray(inputs[k], np.float32) for k in ("b1", "b2", "b3")]
    gs = [np.asarray(inputs[k], np.float32) for k in ("g1", "be1", "g2", "be2")]

    maps = []
    for ci in range(c.NCORES):
        sl = slice(ci * c.NPC, (ci + 1) * c.NPC)
        m = {"x_c": np.ascontiguousarray(x[sl])}
        for li in range(3):
            W = Ws[li]
            din, dout = c.DIMS[li]
            m[f"W{li + 1}c"] = np.ascontiguousarray(
                W.reshape(din // c.P, c.P, dout).transpose(1, 0, 2).astype(BF16))
            m[f"b{li + 1}c"] = bs[li].reshape(1, -1).astype(BF16)
        for gi, n in enumerate(("g1c", "be1c", "g2c", "be2c")):
            m[n] = np.ascontiguousarray(gs[gi])
        dg = pr.deg[sl]
        m["deg_col"] = np.ascontiguousarray(dg.reshape(c.NT, c.P).T)
        m["deg_row"] = np.ascontiguousarray(dg.reshape(1, c.NPC))
        m["batch_col"] = np.ascontiguousarray(
            batch[sl].reshape(c.NT, c.P).T.astype(BF16))
        m["idx16"] = np.ascontiguousarray(pr.idx16[ci])
        m["dstloc"] = np.ascontiguousarray(pr.dstloc[ci])
        maps.append(m)
    return maps


# ---------------------------------------------------------------------------
# entry points
# ---------------------------------------------------------------------------

_CACHE = {}


def _build(inputs, cfg=None):
    cfg = cfg or Cfg()
    key = (cfg.N, cfg.E, cfg.NG, cfg.NQ, cfg.GROUP)
    if key not in _CACHE:
        pr = prep_edges(cfg, np.asarray(inputs["edge_index"], np.int64),
                        np.asarray(inputs["batch"], np.int64))
        nc = build_program(cfg, pr)
        _CACHE[key] = (cfg, pr, nc)
    cfg, pr, nc = _CACHE[key]
    in_maps = make_in_maps(cfg, pr, inputs)
    return cfg, pr, nc, in_maps


def _run(inputs, trace=False, cfg=None):
    from concourse import bass_utils
    cfg, pr, nc, in_maps = _build(inputs, cfg)
    res = bass_utils.run_bass_kernel_spmd(
        nc, in_maps, core_ids=list(range(cfg.NCORES)), trace=False)
    h = np.concatenate([r["h_out"] for r in res.results], axis=0)
    pooled = res.results[0]["pooled_out"]
    return (h.astype(np.float32), pooled.astype(np.float32)), res


def bench(inputs, iters=10, cfg=None):
    """Build the sharded PJRT executable once, keep inputs device-resident,
    and wall-clock repeated executions. Returns ((h, pooled), best_ns)."""
    import time

    import jax
    from jax.sharding import Mesh, NamedSharding, PartitionSpec
    try:
        from jax.experimental.shard_map import shard_map
    except ImportError:
        from jax.sharding import shard_map
    import concourse.mybir as mybir
    from concourse import bass2jax

    cfg, pr, nc, in_maps = _build(inputs, cfg)
    n_cores = cfg.NCORES
    bass2jax.install_neuronx_cc_hook()

    partition_name = (nc.partition_id_tensor.name
                      if nc.partition_id_tensor else None)
    in_names, out_names, out_avals, zero_outs = [], [], [], []
    for alloc in nc.m.functions[0].allocations:
        if not isinstance(alloc, mybir.MemoryLocationSet):
            continue
        name = alloc.memorylocations[0].name
        if alloc.kind == "ExternalInput":
            if name != partition_name:
                in_names.append(name)
        elif alloc.kind == "ExternalOutput":
            shape = tuple(alloc.tensor_shape)
            dtype = mybir.dt.np(alloc.dtype)
            out_names.append(name)
            out_avals.append(jax.core.ShapedArray(shape, dtype))
            zero_outs.append(np.zeros(shape, dtype))
    n_params = len(in_names)
    all_in_names = in_names + out_names
    if partition_name is not None:
        all_in_names = all_in_names + [partition_name]

    def _exec_once(ins, zeros):
        operands = list(ins) + list(zeros)
        if partition_name is not None:
            operands.append(bass2jax.partition_id_tensor())
        outs = bass2jax._bass_exec_p.bind(
            *operands,
            out_avals=tuple(out_avals),
            in_names=tuple(all_in_names),
            out_names=tuple(out_names),
            lowering_input_output_aliases=(),
            sim_require_finite=True,
            sim_require_nnan=True,
            nc=nc,
        )
        return tuple(outs)

    chain = int(os.environ.get("BENCH_CHAIN", "1"))

    def _body(*args):
        ins, zeros = args[:n_params], args[n_params:]
        outs = _exec_once(ins, zeros)
        for _ in range(chain - 1):
            outs = _exec_once(ins, outs)
        return outs

    devices = jax.devices()[:n_cores]
    mesh = Mesh(np.asarray(devices), ("core",))
    spec = PartitionSpec("core")
    sharded = jax.jit(
        shard_map(_body, mesh=mesh, in_specs=(spec,) * (n_params + len(out_names)),
                  out_specs=(spec,) * len(out_names), check_rep=False),
        keep_unused=True)
    sh = NamedSharding(mesh, spec)
    concat_in = [
        jax.device_put(
            np.concatenate([np.asarray(in_maps[c][n]) for c in range(n_cores)],
                           axis=0), sh)
        for n in in_names
    ]
    concat_zeros = [
        jax.device_put(np.zeros((n_cores * z.shape[0], *z.shape[1:]), z.dtype), sh)
        for z in zero_outs
    ]
    out = sharded(*concat_in, *concat_zeros)
    jax.block_until_ready(out)
    times = []
    for _ in range(iters):
        t0 = time.perf_counter()
        out = sharded(*concat_in, *concat_zeros)
        jax.block_until_ready(out)
        times.append(time.perf_counter() - t0)
    best_ns = int(min(times) * 1e9)
    outs = {n: np.asarray(out[i]) for i, n in enumerate(out_names)}
    h = outs["h_out"].reshape(n_cores, cfg.NPC, cfg.D_OUT).reshape(-1, cfg.D_OUT)
    pooled = outs["pooled_out"].reshape(n_cores, cfg.NG, cfg.D_OUT)[0]
    print("bench times (ms):", [f"{t * 1e3:.3f}" for t in times])
    return (h.astype(np.float32), pooled.astype(np.float32)), best_ns


def kernel(**inputs):
    (h, pooled), _ = _run(inputs, trace=False)
    return h, pooled
